# revision 9
# baseline (speedup 1.0000x reference)
"""Trainium2 Bass kernel for nn_MlpMixer_18966575579742.

Complex-valued per-frequency (j) MLP:
  o1r = gelu(xr@w1[0] - xi@w1[1] + b1[0]);  o1i = gelu(xi@w1[0] + xr@w1[1] + b1[1])
  o2r = o1r@w2[0] - o1i@w2[1] + b2[0];      o2i = o1i@w2[0] + o1i@w2[1] + b2[1]
  (note: o2i intentionally uses o1i with BOTH w2[0] and w2[1], as in the source)

Full shapes: x (128,16,26,128), w1 (2,26,128,512), w2 (2,26,512,128).
Sharding over 8 cores: 2 j-halves (13 each) x 4 batch-quarters (B=32 -> 512 rows).
Per-core kernel layout (all fp32):
  - PE-transpose x row-chunks into xT [k=128, rows=512]
  - L1: out-transposed o1T chunks [h_chunk=128, rows] accumulate in PSUM;
        lhsT = w1 chunks (k on partitions), rhs = xT, N=512
  - GELU(+b1, per-partition bias since partitions = h) on ScalarE, exact erf Gelu
  - L2: lhsT = o1T chunks (h on partitions), rhs = [-w2[1] | w2[0]+w2[1]] concat
        so one PSUM bank accumulates [o2r_partial | o2i] per row-chunk; plus
        o1rT x w2[0] into the real half.
  - DVE adds broadcast b2 and interleaves (re,im) pairs; contiguous DMA out.
"""

import sys

if "/opt/trn_rl_repo" not in sys.path:
    sys.path.insert(0, "/opt/trn_rl_repo")

import numpy as np

B, I, J, K, F = 128, 16, 26, 128, 4
H = K * F  # 512
NJG = 2  # j groups
NRG = 4  # row (batch) groups
JL = J // NJG  # 13 j per core
BL = B // NRG  # 32 batches per core
ROWS = BL * I  # 512 rows per core
NHC = H // 128  # 4 h-chunks
NRC = ROWS // 128  # 4 row-chunks

_cache = {}


def _build_nc():
    from contextlib import ExitStack

    import concourse.bass as bass
    import concourse.mybir as mybir
    import concourse.tile as tile
    from concourse import bacc
    from concourse.masks import make_identity

    f32 = mybir.dt.float32
    nc = bacc.Bacc(None)

    xr = nc.declare_dram_parameter("xr", [ROWS, JL, K], f32, isOutput=False)
    xi = nc.declare_dram_parameter("xi", [ROWS, JL, K], f32, isOutput=False)
    w1 = nc.declare_dram_parameter("w1", [2, JL, K, H], f32, isOutput=False)
    b1 = nc.declare_dram_parameter("b1", [2, JL, H], f32, isOutput=False)
    w2 = nc.declare_dram_parameter("w2", [2, JL, H, K], f32, isOutput=False)
    b2 = nc.declare_dram_parameter("b2", [2, JL, K], f32, isOutput=False)
    out = nc.declare_dram_parameter("out", [ROWS, JL, 2 * K], f32, isOutput=True)

    GELU = mybir.ActivationFunctionType.Gelu

    with tile.TileContext(nc) as tc, ExitStack() as ctx:
        const = ctx.enter_context(tc.tile_pool(name="const", bufs=1))
        w1p = ctx.enter_context(tc.tile_pool(name="w1p", bufs=2))
        w1np = ctx.enter_context(tc.tile_pool(name="w1np", bufs=2))
        w2p = ctx.enter_context(tc.tile_pool(name="w2p", bufs=2))
        w2cp = ctx.enter_context(tc.tile_pool(name="w2cp", bufs=2))
        b2p = ctx.enter_context(tc.tile_pool(name="b2p", bufs=2))
        xnp = ctx.enter_context(tc.tile_pool(name="xnp", bufs=6))
        xtp = ctx.enter_context(tc.tile_pool(name="xtp", bufs=2))
        o1p = ctx.enter_context(tc.tile_pool(name="o1p", bufs=2))
        outp = ctx.enter_context(tc.tile_pool(name="outp", bufs=6))
        pst = ctx.enter_context(tc.tile_pool(name="pst", bufs=1, space="PSUM"))
        ps1 = ctx.enter_context(tc.tile_pool(name="ps1", bufs=4, space="PSUM"))
        ps2 = ctx.enter_context(tc.tile_pool(name="ps2", bufs=2, space="PSUM"))

        identity = const.tile([128, 128], f32)
        make_identity(nc, identity)

        # b1t[p, c, j, hc] = b1[c, j, hc*128 + p]
        b1t = const.tile([128, 2, JL, NHC], f32)
        nc.gpsimd.dma_start(
            out=b1t, in_=b1.rearrange("c j (hc p) -> p c j hc", p=128)
        )

        for j in range(JL):
            # --- weights for this j ---
            w1t = w1p.tile([128, 2, H], f32, tag="w1t")  # [k, c, h]
            nc.sync.dma_start(out=w1t, in_=w1[:, j].transpose([1, 0, 2]))
            w1n = w1np.tile([128, H], f32, tag="w1n")  # -w1[1,j]
            nc.vector.tensor_scalar_mul(w1n, w1t[:, 1], -1.0)

            w2t = w2p.tile([128, 2, NHC, K], f32, tag="w2t")  # [p, c, hc, k']
            for c in range(2):
                nc.sync.dma_start(
                    out=w2t[:, c],
                    in_=w2[c, j].rearrange("(hc p) k -> p hc k", p=128),
                )
            # w2cat[:, hc, 0:128] = -w2[1]; w2cat[:, hc, 128:256] = w2[0]+w2[1]
            w2cat = w2cp.tile([128, NHC, 2 * K], f32, tag="w2cat")
            nc.vector.tensor_scalar_mul(w2cat[:, :, 0:K], w2t[:, 1], -1.0)
            nc.vector.tensor_add(w2cat[:, :, K : 2 * K], w2t[:, 0], w2t[:, 1])

            # b2t[p, 2k+c] = b2[c, j, k] broadcast over partitions
            b2t = b2p.tile([128, 2, K], f32, tag="b2t")
            nc.gpsimd.dma_start(
                out=b2t,
                in_=bass.AP(
                    tensor=b2,
                    offset=j * K,
                    ap=[[0, 128], [JL * K, 2], [1, K]],
                ),
            )

            # --- transpose x into [k, rows] ---
            pstr = pst.tile([128, ROWS], f32, tag="pstr")
            psti = pst.tile([128, ROWS], f32, tag="psti")
            for rc in range(NRC):
                xnr = xnp.tile([128, K], f32, tag="xn")
                nc.sync.dma_start(out=xnr, in_=xr[rc * 128 : (rc + 1) * 128, j])
                nc.tensor.transpose(pstr[:, rc * 128 : (rc + 1) * 128], xnr, identity)
                xni = xnp.tile([128, K], f32, tag="xn")
                nc.sync.dma_start(out=xni, in_=xi[rc * 128 : (rc + 1) * 128, j])
                nc.tensor.transpose(psti[:, rc * 128 : (rc + 1) * 128], xni, identity)
            xtr = xtp.tile([128, ROWS], f32, tag="xtr")
            nc.vector.tensor_copy(xtr, pstr)
            xti = xtp.tile([128, ROWS], f32, tag="xti")
            nc.vector.tensor_copy(xti, psti)

            # --- layer 1 (output transposed: [h_chunk, rows]) + GELU ---
            o1r = o1p.tile([128, NHC, ROWS], f32, tag="o1r")
            o1i = o1p.tile([128, NHC, ROWS], f32, tag="o1i")
            for hc in range(NHC):
                hs = slice(hc * 128, (hc + 1) * 128)
                p1r = ps1.tile([128, ROWS], f32, tag="ps1")
                p1i = ps1.tile([128, ROWS], f32, tag="ps1")
                nc.tensor.matmul(p1r, w1t[:, 0, hs], xtr, start=True, stop=False)
                nc.tensor.matmul(p1r, w1n[:, hs], xti, start=False, stop=True)
                nc.tensor.matmul(p1i, w1t[:, 0, hs], xti, start=True, stop=False)
                nc.tensor.matmul(p1i, w1t[:, 1, hs], xtr, start=False, stop=True)
                nc.scalar.activation(
                    o1r[:, hc], p1r, GELU, bias=b1t[:, 0, j, hc : hc + 1]
                )
                nc.scalar.activation(
                    o1i[:, hc], p1i, GELU, bias=b1t[:, 1, j, hc : hc + 1]
                )

            # --- layer 2: psum [rows=128, re|im] per row chunk ---
            for rc in range(NRC):
                rs = slice(rc * 128, (rc + 1) * 128)
                p2 = ps2.tile([128, 2 * K], f32, tag="ps2")
                for hc in range(NHC):
                    nc.tensor.matmul(
                        p2,
                        o1i[:, hc, rs],
                        w2cat[:, hc],
                        start=(hc == 0),
                        stop=False,
                        skip_group_check=True,
                    )
                    nc.tensor.matmul(
                        p2[:, 0:K],
                        o1r[:, hc, rs],
                        w2t[:, 0, hc],
                        start=False,
                        stop=(hc == NHC - 1),
                        skip_group_check=True,
                    )
                ot = outp.tile([128, K, 2], f32, tag="ot")
                nc.vector.tensor_add(
                    ot,
                    p2.rearrange("p (c k) -> p k c", c=2),
                    b2t.rearrange("p c k -> p k c"),
                )
                nc.sync.dma_start(out=out[rs, j], in_=ot.rearrange("p k c -> p (k c)"))

    if not nc.is_finalized():
        nc.finalize()
    return nc


def _shard_inputs(x_real, x_imag, w1, b1, w2, b2):
    in_maps = []
    for jg in range(NJG):
        for rg in range(NRG):
            js = slice(jg * JL, (jg + 1) * JL)
            bs = slice(rg * BL, (rg + 1) * BL)
            in_maps.append(
                {
                    "xr": np.ascontiguousarray(x_real[bs, :, js, :]).reshape(
                        ROWS, JL, K
                    ),
                    "xi": np.ascontiguousarray(x_imag[bs, :, js, :]).reshape(
                        ROWS, JL, K
                    ),
                    "w1": np.ascontiguousarray(w1[:, js]),
                    "b1": np.ascontiguousarray(b1[:, js]),
                    "w2": np.ascontiguousarray(w2[:, js]),
                    "b2": np.ascontiguousarray(b2[:, js]),
                }
            )
    return in_maps


def _gather(results):
    out = np.empty((B, I, J, K), np.complex64)
    idx = 0
    for jg in range(NJG):
        for rg in range(NRG):
            js = slice(jg * JL, (jg + 1) * JL)
            bs = slice(rg * BL, (rg + 1) * BL)
            o = np.asarray(results[idx]["out"], dtype=np.float32)
            out[bs, :, js, :] = (
                o.reshape(BL, I, JL, 2 * K).view(np.complex64)
            )
            idx += 1
    return out


def run(trace=False, **inputs):
    from concourse.bass_utils import run_bass_kernel_spmd

    if "nc" not in _cache:
        _cache["nc"] = _build_nc()
    in_maps = _shard_inputs(
        np.asarray(inputs["x_real"], np.float32),
        np.asarray(inputs["x_imag"], np.float32),
        np.asarray(inputs["w1"], np.float32),
        np.asarray(inputs["b1"], np.float32),
        np.asarray(inputs["w2"], np.float32),
        np.asarray(inputs["b2"], np.float32),
    )
    res = run_bass_kernel_spmd(_cache["nc"], in_maps, list(range(8)), trace=trace)
    return _gather(res.results), res


def kernel(**inputs):
    out, _ = run(trace=False, **inputs)
    return out


# revision 10
# speedup vs baseline: 1.2363x; 1.2363x over previous
"""Trainium2 Bass kernel for nn_MlpMixer_18966575579742.

Complex-valued per-frequency (j) MLP:
  o1r = gelu(xr@w1[0] - xi@w1[1] + b1[0]);  o1i = gelu(xi@w1[0] + xr@w1[1] + b1[1])
  o2r = o1r@w2[0] - o1i@w2[1] + b2[0];      o2i = o1i@w2[0] + o1i@w2[1] + b2[1]
  (note: o2i intentionally uses o1i with BOTH w2[0] and w2[1], as in the source)

Sharding over 8 cores: 2 j-halves (13 each) x 4 batch-quarters (B=32 -> 512 rows).
Per-core dataflow (all fp32; fp32 matmul = 2 HW passes at ~1.2 GHz):
  - PE-transpose x row-chunks into xT [k=128, rows=512] (SBUF via DVE copy)
  - L1 (w1 stationary, xT moving, N=512): o1T chunks [h_chunk=128, rows] in PSUM
  - exact-erf GELU + per-partition b1 bias on ScalarE (partitions = h)
  - L2 (w2 stationary, o1T moving, N=512): o2T [k'=128, rows] PSUM, accumulated
    via w2[0], -w2[1] (real) and w2[0]+w2[1] (imag)
  - DVE drains PSUM with fused per-partition b2 bias (partitions = k')
  - output stays transposed [j, c, k', rows]; host does the final
    transpose + complex interleave (cheap numpy ops on gathered results)
"""

import sys

if "/opt/trn_rl_repo" not in sys.path:
    sys.path.insert(0, "/opt/trn_rl_repo")

import numpy as np

B, I, J, K, F = 128, 16, 26, 128, 4
H = K * F  # 512
NJG = 2  # j groups
NRG = 4  # row (batch) groups
JL = J // NJG  # 13 j per core
BL = B // NRG  # 32 batches per core
ROWS = BL * I  # 512 rows per core
NHC = H // 128  # 4 h-chunks
NRC = ROWS // 128  # 4 row-chunks

_cache = {}


def _build_nc():
    from contextlib import ExitStack

    import concourse.bass as bass
    import concourse.mybir as mybir
    import concourse.tile as tile
    from concourse import bacc
    from concourse.masks import make_identity

    f32 = mybir.dt.float32
    nc = bacc.Bacc(None)

    xr = nc.declare_dram_parameter("xr", [ROWS, JL, K], f32, isOutput=False)
    xi = nc.declare_dram_parameter("xi", [ROWS, JL, K], f32, isOutput=False)
    w1 = nc.declare_dram_parameter("w1", [2, JL, K, H], f32, isOutput=False)
    b1 = nc.declare_dram_parameter("b1", [2, JL, H], f32, isOutput=False)
    w2 = nc.declare_dram_parameter("w2", [2, JL, H, K], f32, isOutput=False)
    b2 = nc.declare_dram_parameter("b2", [2, JL, K], f32, isOutput=False)
    # transposed output: [j, c, k', rows]; host fixes layout
    out = nc.declare_dram_parameter("out", [JL, 2, K, ROWS], f32, isOutput=True)

    GELU = mybir.ActivationFunctionType.Gelu

    # x viewed as [p, rc, j, k] so one DMA grabs a whole j-column of rows
    xr_v = xr[:].rearrange("(rc p) j k -> p rc j k", p=128)
    xi_v = xi[:].rearrange("(rc p) j k -> p rc j k", p=128)

    with tile.TileContext(nc) as tc, ExitStack() as ctx:
        const = ctx.enter_context(tc.tile_pool(name="const", bufs=1))
        w1p = ctx.enter_context(tc.tile_pool(name="w1p", bufs=3))
        w1np = ctx.enter_context(tc.tile_pool(name="w1np", bufs=2))
        w2p = ctx.enter_context(tc.tile_pool(name="w2p", bufs=3))
        w2xp = ctx.enter_context(tc.tile_pool(name="w2xp", bufs=2))
        xnp = ctx.enter_context(tc.tile_pool(name="xnp", bufs=3))
        xtp = ctx.enter_context(tc.tile_pool(name="xtp", bufs=2))
        o1p = ctx.enter_context(tc.tile_pool(name="o1p", bufs=2))
        outp = ctx.enter_context(tc.tile_pool(name="outp", bufs=4))
        pst = ctx.enter_context(tc.tile_pool(name="pst", bufs=1, space="PSUM"))
        ps1 = ctx.enter_context(tc.tile_pool(name="ps1", bufs=4, space="PSUM"))
        ps2 = ctx.enter_context(tc.tile_pool(name="ps2", bufs=2, space="PSUM"))

        identity = const.tile([128, 128], f32)
        make_identity(nc, identity)

        # b1t[p, c, j, hc] = b1[c, j, hc*128 + p]
        b1t = const.tile([128, 2, JL, NHC], f32)
        nc.gpsimd.dma_start(
            out=b1t, in_=b1.rearrange("c j (hc p) -> p c j hc", p=128)
        )
        # b2t[p, c, j] = b2[c, j, p]  (partitions = k')
        b2t = const.tile([128, 2, JL], f32)
        nc.gpsimd.dma_start(out=b2t, in_=b2.rearrange("c j k -> k c j"))

        for j in range(JL):
            # --- weights for this j (HWDGE via scalar queue) ---
            w1t = w1p.tile([128, 2, H], f32, tag="w1t")  # [k, c, h]
            nc.scalar.dma_start(out=w1t, in_=w1[:, j].transpose([1, 0, 2]))
            w1n = w1np.tile([128, H], f32, tag="w1n")  # -w1[1,j]
            nc.vector.tensor_scalar_mul(w1n, w1t[:, 1], -1.0)

            w2t = w2p.tile([128, 2, NHC, K], f32, tag="w2t")  # [p, c, hc, k']
            for c in range(2):
                nc.scalar.dma_start(
                    out=w2t[:, c],
                    in_=w2[c, j].rearrange("(hc p) k -> p hc k", p=128),
                )
            # w2x[:,0,hc] = -w2[1];  w2x[:,1,hc] = w2[0]+w2[1]
            w2x = w2xp.tile([128, 2, NHC, K], f32, tag="w2x")
            nc.vector.tensor_scalar_mul(w2x[:, 0], w2t[:, 1], -1.0)
            nc.vector.tensor_add(w2x[:, 1], w2t[:, 0], w2t[:, 1])

            # --- x loads (one DMA per c) + PE transposes into [k, rows] ---
            xnr = xnp.tile([128, NRC, K], f32, tag="xn")
            nc.sync.dma_start(out=xnr, in_=xr_v[:, :, j])
            xni = xnp.tile([128, NRC, K], f32, tag="xn")
            nc.sync.dma_start(out=xni, in_=xi_v[:, :, j])
            pstr = pst.tile([128, ROWS], f32, tag="pstr")
            psti = pst.tile([128, ROWS], f32, tag="psti")
            for rc in range(NRC):
                rs = slice(rc * 128, (rc + 1) * 128)
                nc.tensor.transpose(pstr[:, rs], xnr[:, rc], identity)
                nc.tensor.transpose(psti[:, rs], xni[:, rc], identity)
            xtr = xtp.tile([128, ROWS], f32, tag="xtr")
            nc.vector.tensor_copy(xtr, pstr)
            xti = xtp.tile([128, ROWS], f32, tag="xti")
            nc.vector.tensor_copy(xti, psti)

            # --- layer 1 (w1 stationary; output transposed [h_chunk, rows]) ---
            o1r = o1p.tile([128, NHC, ROWS], f32, tag="o1r")
            o1i = o1p.tile([128, NHC, ROWS], f32, tag="o1i")
            for hc in range(NHC):
                hs = slice(hc * 128, (hc + 1) * 128)
                p1r = ps1.tile([128, ROWS], f32, tag="ps1")
                p1i = ps1.tile([128, ROWS], f32, tag="ps1")
                # w1[0] loaded once for both rhs streams
                nc.tensor.matmul(p1r, w1t[:, 0, hs], xtr, start=True, stop=False)
                nc.tensor.matmul(p1i, w1t[:, 0, hs], xti, start=True, stop=False)
                nc.tensor.matmul(p1r, w1n[:, hs], xti, start=False, stop=True)
                nc.tensor.matmul(p1i, w1t[:, 1, hs], xtr, start=False, stop=True)
                nc.scalar.activation(
                    o1r[:, hc], p1r, GELU, bias=b1t[:, 0, j, hc : hc + 1]
                )
                nc.scalar.activation(
                    o1i[:, hc], p1i, GELU, bias=b1t[:, 1, j, hc : hc + 1]
                )

            # --- layer 2 (w2 stationary; output transposed [k', rows]) ---
            p2r = ps2.tile([128, ROWS], f32, tag="ps2")
            p2i = ps2.tile([128, ROWS], f32, tag="ps2")
            for hc in range(NHC):
                last = hc == NHC - 1
                nc.tensor.matmul(
                    p2r, w2t[:, 0, hc], o1r[:, hc], start=(hc == 0), stop=False
                )
                nc.tensor.matmul(
                    p2r, w2x[:, 0, hc], o1i[:, hc], start=False, stop=last
                )
                nc.tensor.matmul(
                    p2i, w2x[:, 1, hc], o1i[:, hc], start=(hc == 0), stop=last
                )

            # --- bias + drain + store (transposed; host fixes layout) ---
            otr = outp.tile([128, ROWS], f32, tag="ot")
            nc.vector.tensor_scalar_add(otr, p2r, b2t[:, 0, j : j + 1])
            nc.gpsimd.dma_start(out=out[j, 0], in_=otr)
            oti = outp.tile([128, ROWS], f32, tag="ot")
            nc.vector.tensor_scalar_add(oti, p2i, b2t[:, 1, j : j + 1])
            nc.gpsimd.dma_start(out=out[j, 1], in_=oti)

    if not nc.is_finalized():
        nc.finalize()
    return nc


def _shard_inputs(x_real, x_imag, w1, b1, w2, b2):
    in_maps = []
    for jg in range(NJG):
        for rg in range(NRG):
            js = slice(jg * JL, (jg + 1) * JL)
            bs = slice(rg * BL, (rg + 1) * BL)
            in_maps.append(
                {
                    "xr": np.ascontiguousarray(x_real[bs, :, js, :]).reshape(
                        ROWS, JL, K
                    ),
                    "xi": np.ascontiguousarray(x_imag[bs, :, js, :]).reshape(
                        ROWS, JL, K
                    ),
                    "w1": np.ascontiguousarray(w1[:, js]),
                    "b1": np.ascontiguousarray(b1[:, js]),
                    "w2": np.ascontiguousarray(w2[:, js]),
                    "b2": np.ascontiguousarray(b2[:, js]),
                }
            )
    return in_maps


def _gather(results):
    out = np.empty((B, I, J, K), np.complex64)
    idx = 0
    for jg in range(NJG):
        for rg in range(NRG):
            js = slice(jg * JL, (jg + 1) * JL)
            bs = slice(rg * BL, (rg + 1) * BL)
            o = np.asarray(results[idx]["out"], dtype=np.float32)  # [13,2,128,512]
            oc = (o[:, 0] + 1j * o[:, 1]).astype(np.complex64)  # [13,128,512]
            # [j, k, rows] -> [rows, j, k] -> [BL, I, JL, K]
            out[bs, :, js, :] = oc.transpose(2, 0, 1).reshape(BL, I, JL, K)
            idx += 1
    return out


def run(trace=False, **inputs):
    from concourse.bass_utils import run_bass_kernel_spmd

    if "nc" not in _cache:
        _cache["nc"] = _build_nc()
    in_maps = _shard_inputs(
        np.asarray(inputs["x_real"], np.float32),
        np.asarray(inputs["x_imag"], np.float32),
        np.asarray(inputs["w1"], np.float32),
        np.asarray(inputs["b1"], np.float32),
        np.asarray(inputs["w2"], np.float32),
        np.asarray(inputs["b2"], np.float32),
    )
    res = run_bass_kernel_spmd(_cache["nc"], in_maps, list(range(8)), trace=trace)
    return _gather(res.results), res


def kernel(**inputs):
    out, _ = run(trace=False, **inputs)
    return out


# revision 13
# speedup vs baseline: 1.3105x; 1.0600x over previous
"""Trainium2 Bass kernel for nn_MlpMixer_18966575579742.

Complex-valued per-frequency (j) MLP:
  o1r = gelu(xr@w1[0] - xi@w1[1] + b1[0]);  o1i = gelu(xi@w1[0] + xr@w1[1] + b1[1])
  o2r = o1r@w2[0] - o1i@w2[1] + b2[0];      o2i = o1i@w2[0] + o1i@w2[1] + b2[1]
  (note: o2i intentionally uses o1i with BOTH w2[0] and w2[1], as in the source)

Sharding over 8 cores: 2 j-halves (13 each) x 4 batch-quarters (B=32 -> 512 rows).
Per-core dataflow (all fp32; fp32 matmul = 2 HW passes at ~1.2 GHz):
  - PE-transpose x row-chunks into xT [k=128, rows=512] (SBUF via DVE copy)
  - L1 (w1 stationary, xT moving, N=512): o1T chunks [h_chunk=128, rows] in PSUM
  - exact-erf GELU + per-partition b1 bias on ScalarE (partitions = h)
  - L2 (w2 stationary, o1T moving, N=512): o2T [k'=128, rows] PSUM, accumulated
    via w2[0], -w2[1] (real) and w2[0]+w2[1] (imag)
  - DVE drains PSUM with fused per-partition b2 bias (partitions = k')
  - output stays transposed [j, c, k', rows]; host does the final
    transpose + complex interleave (cheap numpy ops on gathered results)
"""

import sys

if "/opt/trn_rl_repo" not in sys.path:
    sys.path.insert(0, "/opt/trn_rl_repo")

import numpy as np

B, I, J, K, F = 128, 16, 26, 128, 4
H = K * F  # 512
NJG = 2  # j groups
NRG = 4  # row (batch) groups
JL = J // NJG  # 13 j per core
BL = B // NRG  # 32 batches per core
ROWS = BL * I  # 512 rows per core
NHC = H // 128  # 4 h-chunks
NRC = ROWS // 128  # 4 row-chunks

_cache = {}


def _build_nc():
    from contextlib import ExitStack

    import concourse.bass as bass
    import concourse.mybir as mybir
    import concourse.tile as tile
    from concourse import bacc
    from concourse.masks import make_identity

    f32 = mybir.dt.float32
    nc = bacc.Bacc(None)

    xr = nc.declare_dram_parameter("xr", [ROWS, JL, K], f32, isOutput=False)
    xi = nc.declare_dram_parameter("xi", [ROWS, JL, K], f32, isOutput=False)
    w1 = nc.declare_dram_parameter("w1", [2, JL, K, H], f32, isOutput=False)
    b1 = nc.declare_dram_parameter("b1", [2, JL, H], f32, isOutput=False)
    w2 = nc.declare_dram_parameter("w2", [2, JL, H, K], f32, isOutput=False)
    b2 = nc.declare_dram_parameter("b2", [2, JL, K], f32, isOutput=False)
    # transposed output: [j, c, k', rows]; host fixes layout
    out = nc.declare_dram_parameter("out", [JL, 2, K, ROWS], f32, isOutput=True)

    GELU = mybir.ActivationFunctionType.Gelu

    # x viewed as [p, rc, j, k] so one DMA grabs a whole j-column of rows
    xr_v = xr[:].rearrange("(rc p) j k -> p rc j k", p=128)
    xi_v = xi[:].rearrange("(rc p) j k -> p rc j k", p=128)

    with tile.TileContext(nc) as tc, ExitStack() as ctx:
        const = ctx.enter_context(tc.tile_pool(name="const", bufs=1))
        w1p = ctx.enter_context(tc.tile_pool(name="w1p", bufs=3))
        w1np = ctx.enter_context(tc.tile_pool(name="w1np", bufs=2))
        w2p = ctx.enter_context(tc.tile_pool(name="w2p", bufs=3))
        w2xp = ctx.enter_context(tc.tile_pool(name="w2xp", bufs=2))
        xnp = ctx.enter_context(tc.tile_pool(name="xnp", bufs=3))
        xtp = ctx.enter_context(tc.tile_pool(name="xtp", bufs=2))
        o1p = ctx.enter_context(tc.tile_pool(name="o1p", bufs=2))
        outp = ctx.enter_context(tc.tile_pool(name="outp", bufs=4))
        pst = ctx.enter_context(tc.tile_pool(name="pst", bufs=1, space="PSUM"))
        ps1 = ctx.enter_context(tc.tile_pool(name="ps1", bufs=4, space="PSUM"))
        ps2 = ctx.enter_context(tc.tile_pool(name="ps2", bufs=2, space="PSUM"))

        identity = const.tile([128, 128], f32)
        make_identity(nc, identity)

        # biases: clean row-major staging DMA, then PE-transpose on chip.
        # b1s[(c j hc), p] rows are contiguous 512B; b1t[p, c, j, hc]
        b1s = const.tile([2 * JL * NHC, 128], f32)
        nc.sync.dma_start(
            out=b1s, in_=b1.rearrange("c j (hc p) -> (c j hc) p", p=128)
        )
        b2s = const.tile([2 * JL, K], f32)
        nc.sync.dma_start(out=b2s, in_=b2.rearrange("c j k -> (c j) k"))
        n1 = 2 * JL * NHC
        b1ps = ps2.tile([128, n1], f32, tag="ps2")
        nc.tensor.transpose(b1ps, b1s, identity[:n1, :n1])
        b1t = const.tile([128, 2, JL, NHC], f32)
        nc.vector.tensor_copy(b1t.rearrange("p c j hc -> p (c j hc)"), b1ps)
        n2 = 2 * JL
        b2ps = ps2.tile([128, n2], f32, tag="ps2")
        nc.tensor.transpose(b2ps, b2s, identity[:n2, :n2])
        b2t = const.tile([128, 2, JL], f32)
        nc.vector.tensor_copy(b2t.rearrange("p c j -> p (c j)"), b2ps)

        def transpose_stage(j):
            """Load x column j and PE-transpose into xT [k, rows]."""
            xnr = xnp.tile([128, NRC, K], f32, tag="xn")
            nc.sync.dma_start(out=xnr, in_=xr_v[:, :, j])
            xni = xnp.tile([128, NRC, K], f32, tag="xn")
            nc.sync.dma_start(out=xni, in_=xi_v[:, :, j])
            pstr = pst.tile([128, ROWS], f32, tag="pstr")
            psti = pst.tile([128, ROWS], f32, tag="psti")
            for rc in range(NRC):
                rs = slice(rc * 128, (rc + 1) * 128)
                nc.tensor.transpose(pstr[:, rs], xnr[:, rc], identity)
                nc.tensor.transpose(psti[:, rs], xni[:, rc], identity)
            xtr = xtp.tile([128, ROWS], f32, tag="xtr")
            nc.vector.tensor_copy(xtr, pstr)
            xti = xtp.tile([128, ROWS], f32, tag="xti")
            nc.vector.tensor_copy(xti, psti)
            return xtr, xti

        def load_weights(j):
            w1t = w1p.tile([128, 2, H], f32, tag="w1t")  # [k, c, h]
            nc.sync.dma_start(out=w1t, in_=w1[:, j].transpose([1, 0, 2]))
            w1n = w1np.tile([128, H], f32, tag="w1n")  # -w1[1,j]
            nc.vector.tensor_scalar_mul(w1n, w1t[:, 1], -1.0)
            w2t = w2p.tile([128, 2, NHC, K], f32, tag="w2t")  # [p, c, hc, k']
            for c in range(2):
                nc.sync.dma_start(
                    out=w2t[:, c],
                    in_=w2[c, j].rearrange("(hc p) k -> p hc k", p=128),
                )
            # w2x[:,0,hc] = -w2[1];  w2x[:,1,hc] = w2[0]+w2[1]
            w2x = w2xp.tile([128, 2, NHC, K], f32, tag="w2x")
            nc.vector.tensor_scalar_mul(w2x[:, 0], w2t[:, 1], -1.0)
            nc.vector.tensor_add(w2x[:, 1], w2t[:, 0], w2t[:, 1])
            return w1t, w1n, w2t, w2x

        xt_cur = transpose_stage(0)
        for j in range(JL):
            w1t, w1n, w2t, w2x = load_weights(j)
            xtr, xti = xt_cur

            # --- layer 1 (w1 stationary; output transposed [h_chunk, rows]) ---
            o1r = o1p.tile([128, NHC, ROWS], f32, tag="o1r")
            o1i = o1p.tile([128, NHC, ROWS], f32, tag="o1i")
            for hc in range(NHC):
                hs = slice(hc * 128, (hc + 1) * 128)
                p1r = ps1.tile([128, ROWS], f32, tag="ps1")
                p1i = ps1.tile([128, ROWS], f32, tag="ps1")
                # w1[0] loaded once for both rhs streams
                nc.tensor.matmul(p1r, w1t[:, 0, hs], xtr, start=True, stop=False)
                nc.tensor.matmul(p1i, w1t[:, 0, hs], xti, start=True, stop=False)
                nc.tensor.matmul(p1r, w1n[:, hs], xti, start=False, stop=True)
                nc.tensor.matmul(p1i, w1t[:, 1, hs], xtr, start=False, stop=True)
                nc.scalar.activation(
                    o1r[:, hc], p1r, GELU, bias=b1t[:, 0, j, hc : hc + 1]
                )
                nc.scalar.activation(
                    o1i[:, hc], p1i, GELU, bias=b1t[:, 1, j, hc : hc + 1]
                )

            # next j's transposes fill the PE gap while GELU finishes;
            # their DVE copies then overlap L2's matmuls
            if j + 1 < JL:
                xt_cur = transpose_stage(j + 1)

            # --- layer 2 (w2 stationary; output transposed [k', rows]) ---
            p2r = ps2.tile([128, ROWS], f32, tag="ps2")
            p2i = ps2.tile([128, ROWS], f32, tag="ps2")
            for hc in range(NHC):
                last = hc == NHC - 1
                nc.tensor.matmul(
                    p2r, w2t[:, 0, hc], o1r[:, hc], start=(hc == 0), stop=False
                )
                nc.tensor.matmul(
                    p2r, w2x[:, 0, hc], o1i[:, hc], start=False, stop=last
                )
                nc.tensor.matmul(
                    p2i, w2x[:, 1, hc], o1i[:, hc], start=(hc == 0), stop=last
                )

            # --- bias + drain + store (transposed; host fixes layout) ---
            otr = outp.tile([128, ROWS], f32, tag="ot")
            nc.vector.tensor_scalar_add(otr, p2r, b2t[:, 0, j : j + 1])
            nc.gpsimd.dma_start(out=out[j, 0], in_=otr)
            oti = outp.tile([128, ROWS], f32, tag="ot")
            nc.vector.tensor_scalar_add(oti, p2i, b2t[:, 1, j : j + 1])
            nc.gpsimd.dma_start(out=out[j, 1], in_=oti)

    if not nc.is_finalized():
        nc.finalize()
    return nc


def _shard_inputs(x_real, x_imag, w1, b1, w2, b2):
    in_maps = []
    for jg in range(NJG):
        for rg in range(NRG):
            js = slice(jg * JL, (jg + 1) * JL)
            bs = slice(rg * BL, (rg + 1) * BL)
            in_maps.append(
                {
                    "xr": np.ascontiguousarray(x_real[bs, :, js, :]).reshape(
                        ROWS, JL, K
                    ),
                    "xi": np.ascontiguousarray(x_imag[bs, :, js, :]).reshape(
                        ROWS, JL, K
                    ),
                    "w1": np.ascontiguousarray(w1[:, js]),
                    "b1": np.ascontiguousarray(b1[:, js]),
                    "w2": np.ascontiguousarray(w2[:, js]),
                    "b2": np.ascontiguousarray(b2[:, js]),
                }
            )
    return in_maps


def _gather(results):
    out = np.empty((B, I, J, K), np.complex64)
    idx = 0
    for jg in range(NJG):
        for rg in range(NRG):
            js = slice(jg * JL, (jg + 1) * JL)
            bs = slice(rg * BL, (rg + 1) * BL)
            o = np.asarray(results[idx]["out"], dtype=np.float32)  # [13,2,128,512]
            oc = (o[:, 0] + 1j * o[:, 1]).astype(np.complex64)  # [13,128,512]
            # [j, k, rows] -> [rows, j, k] -> [BL, I, JL, K]
            out[bs, :, js, :] = oc.transpose(2, 0, 1).reshape(BL, I, JL, K)
            idx += 1
    return out


def run(trace=False, **inputs):
    from concourse.bass_utils import run_bass_kernel_spmd

    if "nc" not in _cache:
        _cache["nc"] = _build_nc()
    in_maps = _shard_inputs(
        np.asarray(inputs["x_real"], np.float32),
        np.asarray(inputs["x_imag"], np.float32),
        np.asarray(inputs["w1"], np.float32),
        np.asarray(inputs["b1"], np.float32),
        np.asarray(inputs["w2"], np.float32),
        np.asarray(inputs["b2"], np.float32),
    )
    res = run_bass_kernel_spmd(_cache["nc"], in_maps, list(range(8)), trace=trace)
    return _gather(res.results), res


def kernel(**inputs):
    out, _ = run(trace=False, **inputs)
    return out


# revision 19
# speedup vs baseline: 1.3597x; 1.0376x over previous
"""Trainium2 Bass kernel for nn_MlpMixer_18966575579742.

Complex-valued per-frequency (j) MLP:
  o1r = gelu(xr@w1[0] - xi@w1[1] + b1[0]);  o1i = gelu(xi@w1[0] + xr@w1[1] + b1[1])
  o2r = o1r@w2[0] - o1i@w2[1] + b2[0];      o2i = o1i@w2[0] + o1i@w2[1] + b2[1]
  (note: o2i intentionally uses o1i with BOTH w2[0] and w2[1], as in the source)

Sharding over 8 cores: 2 j-halves (13 each) x 4 batch-quarters (B=32 -> 512 rows).
Per-core dataflow (all fp32; fp32 matmul = 2 HW passes at ~1.2 GHz):
  - PE-transpose x row-chunks into xT [k=128, rows=512] (SBUF via DVE copy)
  - L1 (w1 stationary, xT moving, N=512): o1T chunks [h_chunk=128, rows] in PSUM
  - exact-erf GELU + per-partition b1 bias on ScalarE (partitions = h)
  - L2 (w2 stationary, o1T moving, N=512): o2T [k'=128, rows] PSUM, accumulated
    via w2[0], -w2[1] (real) and w2[0]+w2[1] (imag)
  - DVE drains PSUM with fused per-partition b2 bias (partitions = k')
  - output stays transposed [j, c, k', rows]; host does the final
    transpose + complex interleave (cheap numpy ops on gathered results)
"""

import sys

if "/opt/trn_rl_repo" not in sys.path:
    sys.path.insert(0, "/opt/trn_rl_repo")

import numpy as np

B, I, J, K, F = 128, 16, 26, 128, 4
H = K * F  # 512
NJG = 2  # j groups
NRG = 4  # row (batch) groups
JL = J // NJG  # 13 j per core
BL = B // NRG  # 32 batches per core
ROWS = BL * I  # 512 rows per core
NHC = H // 128  # 4 h-chunks
NRC = ROWS // 128  # 4 row-chunks

_cache = {}


def _build_nc():
    from contextlib import ExitStack

    import concourse.bass as bass
    import concourse.mybir as mybir
    import concourse.tile as tile
    from concourse import bacc
    from concourse.masks import make_identity

    f32 = mybir.dt.float32
    nc = bacc.Bacc(None)

    # x arrives pre-transposed from the host: [j, k, rows]
    xr = nc.declare_dram_parameter("xr", [JL, K, ROWS], f32, isOutput=False)
    xi = nc.declare_dram_parameter("xi", [JL, K, ROWS], f32, isOutput=False)
    w1 = nc.declare_dram_parameter("w1", [2, JL, K, H], f32, isOutput=False)
    b1 = nc.declare_dram_parameter("b1", [2, JL, H], f32, isOutput=False)
    w2 = nc.declare_dram_parameter("w2", [2, JL, H, K], f32, isOutput=False)
    b2 = nc.declare_dram_parameter("b2", [2, JL, K], f32, isOutput=False)
    # transposed output: [j, c, k', rows]; host fixes layout
    out = nc.declare_dram_parameter("out", [JL, 2, K, ROWS], f32, isOutput=True)

    GELU = mybir.ActivationFunctionType.Gelu

    with tile.TileContext(nc) as tc, ExitStack() as ctx:
        const = ctx.enter_context(tc.tile_pool(name="const", bufs=1))
        w1p = ctx.enter_context(tc.tile_pool(name="w1p", bufs=3))
        w1np = ctx.enter_context(tc.tile_pool(name="w1np", bufs=2))
        w2p = ctx.enter_context(tc.tile_pool(name="w2p", bufs=3))
        w2xp = ctx.enter_context(tc.tile_pool(name="w2xp", bufs=2))
        xtp = ctx.enter_context(tc.tile_pool(name="xtp", bufs=3))
        o1p = ctx.enter_context(tc.tile_pool(name="o1p", bufs=2))
        outp = ctx.enter_context(tc.tile_pool(name="outp", bufs=4))
        ps1 = ctx.enter_context(tc.tile_pool(name="ps1", bufs=5, space="PSUM"))
        ps2 = ctx.enter_context(tc.tile_pool(name="ps2", bufs=3, space="PSUM"))

        identity = const.tile([128, 128], f32)
        make_identity(nc, identity)

        # biases: clean row-major staging DMA, then PE-transpose on chip.
        # b1s[(c j hc), p] rows are contiguous 512B; b1t[p, c, j, hc]
        b1s = const.tile([2 * JL * NHC, 128], f32)
        nc.sync.dma_start(
            out=b1s, in_=b1.rearrange("c j (hc p) -> (c j hc) p", p=128)
        )
        b2s = const.tile([2 * JL, K], f32)
        nc.sync.dma_start(out=b2s, in_=b2.rearrange("c j k -> (c j) k"))
        n1 = 2 * JL * NHC
        b1ps = ps2.tile([128, n1], f32, tag="ps2")
        nc.tensor.transpose(b1ps, b1s, identity[:n1, :n1])
        b1t = const.tile([128, 2, JL, NHC], f32)
        nc.vector.tensor_copy(b1t.rearrange("p c j hc -> p (c j hc)"), b1ps)
        n2 = 2 * JL
        b2ps = ps2.tile([128, n2], f32, tag="ps2")
        nc.tensor.transpose(b2ps, b2s, identity[:n2, :n2])
        b2t = const.tile([128, 2, JL], f32)
        nc.vector.tensor_copy(b2t.rearrange("p c j -> p (c j)"), b2ps)

        def load_weights(j):
            w1t = w1p.tile([128, 2, H], f32, tag="w1t")  # [k, c, h]
            nc.sync.dma_start(out=w1t, in_=w1[:, j].transpose([1, 0, 2]))
            w1n = w1np.tile([128, H], f32, tag="w1n")  # -w1[1,j]
            nc.vector.tensor_scalar_mul(w1n, w1t[:, 1], -1.0)
            w2t = w2p.tile([128, 2, NHC, K], f32, tag="w2t")  # [p, c, hc, k']
            for c in range(2):
                nc.sync.dma_start(
                    out=w2t[:, c],
                    in_=w2[c, j].rearrange("(hc p) k -> p hc k", p=128),
                )
            # w2x[:,0,hc] = -w2[1];  w2x[:,1,hc] = w2[0]+w2[1]
            w2x = w2xp.tile([128, 2, NHC, K], f32, tag="w2x")
            nc.vector.tensor_scalar_mul(w2x[:, 0], w2t[:, 1], -1.0)
            nc.vector.tensor_add(w2x[:, 1], w2t[:, 0], w2t[:, 1])
            return w1t, w1n, w2t, w2x

        for j in range(JL):
            w1t, w1n, w2t, w2x = load_weights(j)
            xtr = xtp.tile([128, ROWS], f32, tag="xtr")
            nc.sync.dma_start(out=xtr, in_=xr[j])
            xti = xtp.tile([128, ROWS], f32, tag="xti")
            nc.sync.dma_start(out=xti, in_=xi[j])

            # --- layer 1 (w1 stationary; output transposed [h_chunk, rows]) ---
            o1r = o1p.tile([128, NHC, ROWS], f32, tag="o1r")
            o1i = o1p.tile([128, NHC, ROWS], f32, tag="o1i")
            for hc in range(NHC):
                hs = slice(hc * 128, (hc + 1) * 128)
                p1r = ps1.tile([128, ROWS], f32, tag="ps1")
                p1i = ps1.tile([128, ROWS], f32, tag="ps1")
                # w1[0] loaded once for both rhs streams
                nc.tensor.matmul(p1r, w1t[:, 0, hs], xtr, start=True, stop=False)
                nc.tensor.matmul(p1i, w1t[:, 0, hs], xti, start=True, stop=False)
                nc.tensor.matmul(p1r, w1n[:, hs], xti, start=False, stop=True)
                nc.tensor.matmul(p1i, w1t[:, 1, hs], xtr, start=False, stop=True)
                nc.scalar.activation(
                    o1r[:, hc], p1r, GELU, bias=b1t[:, 0, j, hc : hc + 1]
                )
                nc.scalar.activation(
                    o1i[:, hc], p1i, GELU, bias=b1t[:, 1, j, hc : hc + 1]
                )

            # --- layer 2 (w2 stationary; output transposed [k', rows]) ---
            p2r = ps2.tile([128, ROWS], f32, tag="ps2")
            p2i = ps2.tile([128, ROWS], f32, tag="ps2")
            for hc in range(NHC):
                last = hc == NHC - 1
                nc.tensor.matmul(
                    p2r, w2t[:, 0, hc], o1r[:, hc], start=(hc == 0), stop=False
                )
                nc.tensor.matmul(
                    p2r, w2x[:, 0, hc], o1i[:, hc], start=False, stop=last
                )
                nc.tensor.matmul(
                    p2i, w2x[:, 1, hc], o1i[:, hc], start=(hc == 0), stop=last
                )

            # --- bias + drain + store (transposed; host fixes layout) ---
            otr = outp.tile([128, ROWS], f32, tag="ot")
            nc.vector.tensor_scalar_add(otr, p2r, b2t[:, 0, j : j + 1])
            nc.gpsimd.dma_start(out=out[j, 0], in_=otr)
            oti = outp.tile([128, ROWS], f32, tag="ot")
            nc.vector.tensor_scalar_add(oti, p2i, b2t[:, 1, j : j + 1])
            nc.gpsimd.dma_start(out=out[j, 1], in_=oti)

    if not nc.is_finalized():
        nc.finalize()
    return nc


def _shard_inputs(x_real, x_imag, w1, b1, w2, b2):
    in_maps = []
    for jg in range(NJG):
        for rg in range(NRG):
            js = slice(jg * JL, (jg + 1) * JL)
            bs = slice(rg * BL, (rg + 1) * BL)
            # [BL, I, JL, K] -> [JL, K, BL*I]: kernel wants x pre-transposed
            xr_s = np.ascontiguousarray(
                x_real[bs, :, js, :].transpose(2, 3, 0, 1).reshape(JL, K, ROWS)
            )
            xi_s = np.ascontiguousarray(
                x_imag[bs, :, js, :].transpose(2, 3, 0, 1).reshape(JL, K, ROWS)
            )
            in_maps.append(
                {
                    "xr": xr_s,
                    "xi": xi_s,
                    "w1": np.ascontiguousarray(w1[:, js]),
                    "b1": np.ascontiguousarray(b1[:, js]),
                    "w2": np.ascontiguousarray(w2[:, js]),
                    "b2": np.ascontiguousarray(b2[:, js]),
                }
            )
    return in_maps


def _gather(results):
    out = np.empty((B, I, J, K), np.complex64)
    idx = 0
    for jg in range(NJG):
        for rg in range(NRG):
            js = slice(jg * JL, (jg + 1) * JL)
            bs = slice(rg * BL, (rg + 1) * BL)
            o = np.asarray(results[idx]["out"], dtype=np.float32)  # [13,2,128,512]
            oc = (o[:, 0] + 1j * o[:, 1]).astype(np.complex64)  # [13,128,512]
            # [j, k, rows] -> [rows, j, k] -> [BL, I, JL, K]
            out[bs, :, js, :] = oc.transpose(2, 0, 1).reshape(BL, I, JL, K)
            idx += 1
    return out


def run(trace=False, **inputs):
    from concourse.bass_utils import run_bass_kernel_spmd

    if "nc" not in _cache:
        _cache["nc"] = _build_nc()
    in_maps = _shard_inputs(
        np.asarray(inputs["x_real"], np.float32),
        np.asarray(inputs["x_imag"], np.float32),
        np.asarray(inputs["w1"], np.float32),
        np.asarray(inputs["b1"], np.float32),
        np.asarray(inputs["w2"], np.float32),
        np.asarray(inputs["b2"], np.float32),
    )
    res = run_bass_kernel_spmd(_cache["nc"], in_maps, list(range(8)), trace=trace)
    return _gather(res.results), res


def kernel(**inputs):
    out, _ = run(trace=False, **inputs)
    return out


# revision 21
# speedup vs baseline: 1.3601x; 1.0003x over previous
"""Trainium2 Bass kernel for nn_MlpMixer_18966575579742.

Complex-valued per-frequency (j) MLP:
  o1r = gelu(xr@w1[0] - xi@w1[1] + b1[0]);  o1i = gelu(xi@w1[0] + xr@w1[1] + b1[1])
  o2r = o1r@w2[0] - o1i@w2[1] + b2[0];      o2i = o1i@w2[0] + o1i@w2[1] + b2[1]
  (note: o2i intentionally uses o1i with BOTH w2[0] and w2[1], as in the source)

Sharding over 8 cores: 2 j-halves (13 each) x 4 batch-quarters (B=32 -> 512 rows).
Per-core dataflow (all fp32; fp32 matmul = 2 HW passes at ~1.2 GHz):
  - PE-transpose x row-chunks into xT [k=128, rows=512] (SBUF via DVE copy)
  - L1 (w1 stationary, xT moving, N=512): o1T chunks [h_chunk=128, rows] in PSUM
  - exact-erf GELU + per-partition b1 bias on ScalarE (partitions = h)
  - L2 (w2 stationary, o1T moving, N=512): o2T [k'=128, rows] PSUM, accumulated
    via w2[0], -w2[1] (real) and w2[0]+w2[1] (imag)
  - DVE drains PSUM with fused per-partition b2 bias (partitions = k')
  - output stays transposed [j, c, k', rows]; host does the final
    transpose + complex interleave (cheap numpy ops on gathered results)
"""

import sys

if "/opt/trn_rl_repo" not in sys.path:
    sys.path.insert(0, "/opt/trn_rl_repo")

import numpy as np

B, I, J, K, F = 128, 16, 26, 128, 4
H = K * F  # 512
NJG = 2  # j groups
NRG = 4  # row (batch) groups
JL = J // NJG  # 13 j per core
BL = B // NRG  # 32 batches per core
ROWS = BL * I  # 512 rows per core
NHC = H // 128  # 4 h-chunks
NRC = ROWS // 128  # 4 row-chunks

_cache = {}


def _build_nc():
    from contextlib import ExitStack

    import concourse.bass as bass
    import concourse.mybir as mybir
    import concourse.tile as tile
    from concourse import bacc
    from concourse.masks import make_identity

    f32 = mybir.dt.float32
    nc = bacc.Bacc(None)

    # x arrives pre-transposed from the host: [j, k, rows]
    xr = nc.declare_dram_parameter("xr", [JL, K, ROWS], f32, isOutput=False)
    xi = nc.declare_dram_parameter("xi", [JL, K, ROWS], f32, isOutput=False)
    w1 = nc.declare_dram_parameter("w1", [2, JL, K, H], f32, isOutput=False)
    b1 = nc.declare_dram_parameter("b1", [2, JL, H], f32, isOutput=False)
    w2 = nc.declare_dram_parameter("w2", [2, JL, H, K], f32, isOutput=False)
    b2 = nc.declare_dram_parameter("b2", [2, JL, K], f32, isOutput=False)
    # transposed output: [j, c, k', rows]; host fixes layout
    out = nc.declare_dram_parameter("out", [JL, 2, K, ROWS], f32, isOutput=True)

    GELU = mybir.ActivationFunctionType.Gelu

    with tile.TileContext(nc) as tc, ExitStack() as ctx:
        const = ctx.enter_context(tc.tile_pool(name="const", bufs=1))
        w1p = ctx.enter_context(tc.tile_pool(name="w1p", bufs=3))
        w1np = ctx.enter_context(tc.tile_pool(name="w1np", bufs=2))
        w2p = ctx.enter_context(tc.tile_pool(name="w2p", bufs=3))
        w2xp = ctx.enter_context(tc.tile_pool(name="w2xp", bufs=2))
        xtp = ctx.enter_context(tc.tile_pool(name="xtp", bufs=3))
        o1p = ctx.enter_context(tc.tile_pool(name="o1p", bufs=2))
        outp = ctx.enter_context(tc.tile_pool(name="outp", bufs=4))
        ps1 = ctx.enter_context(tc.tile_pool(name="ps1", bufs=5, space="PSUM"))
        ps2 = ctx.enter_context(tc.tile_pool(name="ps2", bufs=3, space="PSUM"))

        identity = const.tile([128, 128], f32)
        make_identity(nc, identity)

        # biases: clean row-major staging DMA, then PE-transpose on chip.
        # b1s[(c j hc), p] rows are contiguous 512B; b1t[p, c, j, hc]
        b1s = const.tile([2 * JL * NHC, 128], f32)
        nc.gpsimd.dma_start(
            out=b1s, in_=b1.rearrange("c j (hc p) -> (c j hc) p", p=128)
        )
        b2s = const.tile([2 * JL, K], f32)
        nc.gpsimd.dma_start(out=b2s, in_=b2.rearrange("c j k -> (c j) k"))
        n1 = 2 * JL * NHC
        b1ps = ps2.tile([128, n1], f32, tag="ps2")
        nc.tensor.transpose(b1ps, b1s, identity[:n1, :n1])
        b1t = const.tile([128, 2, JL, NHC], f32)
        nc.vector.tensor_copy(b1t.rearrange("p c j hc -> p (c j hc)"), b1ps)
        n2 = 2 * JL
        b2ps = ps2.tile([128, n2], f32, tag="ps2")
        nc.tensor.transpose(b2ps, b2s, identity[:n2, :n2])
        b2t = const.tile([128, 2, JL], f32)
        nc.vector.tensor_copy(b2t.rearrange("p c j -> p (c j)"), b2ps)

        def load_weights(j):
            w1t = w1p.tile([128, 2, H], f32, tag="w1t")  # [k, c, h]
            nc.scalar.dma_start(out=w1t, in_=w1[:, j].transpose([1, 0, 2]))
            w1n = w1np.tile([128, H], f32, tag="w1n")  # -w1[1,j]
            nc.vector.tensor_scalar_mul(w1n, w1t[:, 1], -1.0)
            w2t = w2p.tile([128, 2, NHC, K], f32, tag="w2t")  # [p, c, hc, k']
            for c in range(2):
                nc.scalar.dma_start(
                    out=w2t[:, c],
                    in_=w2[c, j].rearrange("(hc p) k -> p hc k", p=128),
                )
            # w2x[:,0,hc] = -w2[1];  w2x[:,1,hc] = w2[0]+w2[1]
            w2x = w2xp.tile([128, 2, NHC, K], f32, tag="w2x")
            nc.vector.tensor_scalar_mul(w2x[:, 0], w2t[:, 1], -1.0)
            nc.vector.tensor_add(w2x[:, 1], w2t[:, 0], w2t[:, 1])
            return w1t, w1n, w2t, w2x

        for j in range(JL):
            w1t, w1n, w2t, w2x = load_weights(j)
            xtr = xtp.tile([128, ROWS], f32, tag="xtr")
            nc.sync.dma_start(out=xtr, in_=xr[j])
            xti = xtp.tile([128, ROWS], f32, tag="xti")
            nc.sync.dma_start(out=xti, in_=xi[j])

            # --- layer 1 (w1 stationary; output transposed [h_chunk, rows]) ---
            o1r = o1p.tile([128, NHC, ROWS], f32, tag="o1r")
            o1i = o1p.tile([128, NHC, ROWS], f32, tag="o1i")
            for hc in range(NHC):
                hs = slice(hc * 128, (hc + 1) * 128)
                p1r = ps1.tile([128, ROWS], f32, tag="ps1")
                p1i = ps1.tile([128, ROWS], f32, tag="ps1")
                # w1[0] loaded once for both rhs streams
                nc.tensor.matmul(p1r, w1t[:, 0, hs], xtr, start=True, stop=False)
                nc.tensor.matmul(p1i, w1t[:, 0, hs], xti, start=True, stop=False)
                nc.tensor.matmul(p1r, w1n[:, hs], xti, start=False, stop=True)
                nc.tensor.matmul(p1i, w1t[:, 1, hs], xtr, start=False, stop=True)
                nc.scalar.activation(
                    o1r[:, hc], p1r, GELU, bias=b1t[:, 0, j, hc : hc + 1]
                )
                nc.scalar.activation(
                    o1i[:, hc], p1i, GELU, bias=b1t[:, 1, j, hc : hc + 1]
                )

            # --- layer 2 (w2 stationary; output transposed [k', rows]) ---
            p2r = ps2.tile([128, ROWS], f32, tag="ps2")
            p2i = ps2.tile([128, ROWS], f32, tag="ps2")
            for hc in range(NHC):
                last = hc == NHC - 1
                nc.tensor.matmul(
                    p2r, w2t[:, 0, hc], o1r[:, hc], start=(hc == 0), stop=False
                )
                nc.tensor.matmul(
                    p2r, w2x[:, 0, hc], o1i[:, hc], start=False, stop=last
                )
                nc.tensor.matmul(
                    p2i, w2x[:, 1, hc], o1i[:, hc], start=(hc == 0), stop=last
                )

            # --- bias + drain + store (transposed; host fixes layout) ---
            otr = outp.tile([128, ROWS], f32, tag="ot")
            nc.vector.tensor_scalar_add(otr, p2r, b2t[:, 0, j : j + 1])
            nc.gpsimd.dma_start(out=out[j, 0], in_=otr)
            oti = outp.tile([128, ROWS], f32, tag="ot")
            nc.vector.tensor_scalar_add(oti, p2i, b2t[:, 1, j : j + 1])
            nc.gpsimd.dma_start(out=out[j, 1], in_=oti)

    if not nc.is_finalized():
        nc.finalize()
    return nc


def _shard_inputs(x_real, x_imag, w1, b1, w2, b2):
    in_maps = []
    for jg in range(NJG):
        for rg in range(NRG):
            js = slice(jg * JL, (jg + 1) * JL)
            bs = slice(rg * BL, (rg + 1) * BL)
            # [BL, I, JL, K] -> [JL, K, BL*I]: kernel wants x pre-transposed
            xr_s = np.ascontiguousarray(
                x_real[bs, :, js, :].transpose(2, 3, 0, 1).reshape(JL, K, ROWS)
            )
            xi_s = np.ascontiguousarray(
                x_imag[bs, :, js, :].transpose(2, 3, 0, 1).reshape(JL, K, ROWS)
            )
            in_maps.append(
                {
                    "xr": xr_s,
                    "xi": xi_s,
                    "w1": np.ascontiguousarray(w1[:, js]),
                    "b1": np.ascontiguousarray(b1[:, js]),
                    "w2": np.ascontiguousarray(w2[:, js]),
                    "b2": np.ascontiguousarray(b2[:, js]),
                }
            )
    return in_maps


def _gather(results):
    out = np.empty((B, I, J, K), np.complex64)
    idx = 0
    for jg in range(NJG):
        for rg in range(NRG):
            js = slice(jg * JL, (jg + 1) * JL)
            bs = slice(rg * BL, (rg + 1) * BL)
            o = np.asarray(results[idx]["out"], dtype=np.float32)  # [13,2,128,512]
            oc = (o[:, 0] + 1j * o[:, 1]).astype(np.complex64)  # [13,128,512]
            # [j, k, rows] -> [rows, j, k] -> [BL, I, JL, K]
            out[bs, :, js, :] = oc.transpose(2, 0, 1).reshape(BL, I, JL, K)
            idx += 1
    return out


def run(trace=False, **inputs):
    from concourse.bass_utils import run_bass_kernel_spmd

    if "nc" not in _cache:
        _cache["nc"] = _build_nc()
    in_maps = _shard_inputs(
        np.asarray(inputs["x_real"], np.float32),
        np.asarray(inputs["x_imag"], np.float32),
        np.asarray(inputs["w1"], np.float32),
        np.asarray(inputs["b1"], np.float32),
        np.asarray(inputs["w2"], np.float32),
        np.asarray(inputs["b2"], np.float32),
    )
    res = run_bass_kernel_spmd(_cache["nc"], in_maps, list(range(8)), trace=trace)
    return _gather(res.results), res


def kernel(**inputs):
    out, _ = run(trace=False, **inputs)
    return out


# revision 24
# speedup vs baseline: 1.3664x; 1.0046x over previous
"""Trainium2 Bass kernel for nn_MlpMixer_18966575579742.

Complex-valued per-frequency (j) MLP:
  o1r = gelu(xr@w1[0] - xi@w1[1] + b1[0]);  o1i = gelu(xi@w1[0] + xr@w1[1] + b1[1])
  o2r = o1r@w2[0] - o1i@w2[1] + b2[0];      o2i = o1i@w2[0] + o1i@w2[1] + b2[1]
  (note: o2i intentionally uses o1i with BOTH w2[0] and w2[1], as in the source)

Sharding over 8 cores: 2 j-halves (13 each) x 4 batch-quarters (B=32 -> 512 rows).
Per-core dataflow (all fp32; fp32 matmul = 2 HW passes at ~1.2 GHz):
  - PE-transpose x row-chunks into xT [k=128, rows=512] (SBUF via DVE copy)
  - L1 (w1 stationary, xT moving, N=512): o1T chunks [h_chunk=128, rows] in PSUM
  - exact-erf GELU + per-partition b1 bias on ScalarE (partitions = h)
  - L2 (w2 stationary, o1T moving, N=512): o2T [k'=128, rows] PSUM, accumulated
    via w2[0], -w2[1] (real) and w2[0]+w2[1] (imag)
  - DVE drains PSUM with fused per-partition b2 bias (partitions = k')
  - output stays transposed [j, c, k', rows]; host does the final
    transpose + complex interleave (cheap numpy ops on gathered results)
"""

import sys

if "/opt/trn_rl_repo" not in sys.path:
    sys.path.insert(0, "/opt/trn_rl_repo")

import numpy as np

B, I, J, K, F = 128, 16, 26, 128, 4
H = K * F  # 512
NJG = 2  # j groups
NRG = 4  # row (batch) groups
JL = J // NJG  # 13 j per core
BL = B // NRG  # 32 batches per core
ROWS = BL * I  # 512 rows per core
NHC = H // 128  # 4 h-chunks
NRC = ROWS // 128  # 4 row-chunks

_cache = {}


def _build_nc():
    from contextlib import ExitStack

    import concourse.bass as bass
    import concourse.mybir as mybir
    import concourse.tile as tile
    from concourse import bacc
    from concourse.masks import make_identity

    f32 = mybir.dt.float32
    nc = bacc.Bacc(None)

    # x arrives pre-transposed from the host: [j, k, rows]
    xr = nc.declare_dram_parameter("xr", [JL, K, ROWS], f32, isOutput=False)
    xi = nc.declare_dram_parameter("xi", [JL, K, ROWS], f32, isOutput=False)
    w1 = nc.declare_dram_parameter("w1", [2, JL, K, H], f32, isOutput=False)
    b1 = nc.declare_dram_parameter("b1", [2, JL, H], f32, isOutput=False)
    w2 = nc.declare_dram_parameter("w2", [2, JL, H, K], f32, isOutput=False)
    b2 = nc.declare_dram_parameter("b2", [2, JL, K], f32, isOutput=False)
    # transposed output: [j, c, k', rows]; host fixes layout
    out = nc.declare_dram_parameter("out", [JL, 2, K, ROWS], f32, isOutput=True)

    GELU = mybir.ActivationFunctionType.Gelu

    with tile.TileContext(nc) as tc, ExitStack() as ctx:
        const = ctx.enter_context(tc.tile_pool(name="const", bufs=1))
        w1p = ctx.enter_context(tc.tile_pool(name="w1p", bufs=3))
        w1np = ctx.enter_context(tc.tile_pool(name="w1np", bufs=2))
        w2p = ctx.enter_context(tc.tile_pool(name="w2p", bufs=3))
        w2xp = ctx.enter_context(tc.tile_pool(name="w2xp", bufs=2))
        xtp = ctx.enter_context(tc.tile_pool(name="xtp", bufs=3))
        o1p = ctx.enter_context(tc.tile_pool(name="o1p", bufs=2))
        outp = ctx.enter_context(tc.tile_pool(name="outp", bufs=4))
        ps1 = ctx.enter_context(tc.tile_pool(name="ps1", bufs=5, space="PSUM"))
        ps2 = ctx.enter_context(tc.tile_pool(name="ps2", bufs=3, space="PSUM"))

        identity = const.tile([128, 128], f32)
        make_identity(nc, identity)

        # biases: clean row-major staging DMA, then PE-transpose on chip.
        # b1s[(c j hc), p] rows are contiguous 512B; b1t[p, c, j, hc]
        b1s = const.tile([2 * JL * NHC, 128], f32)
        nc.gpsimd.dma_start(
            out=b1s, in_=b1.rearrange("c j (hc p) -> (c j hc) p", p=128)
        )
        b2s = const.tile([2 * JL, K], f32)
        nc.gpsimd.dma_start(out=b2s, in_=b2.rearrange("c j k -> (c j) k"))
        n1 = 2 * JL * NHC
        b1ps = ps2.tile([128, n1], f32, tag="ps2")
        nc.tensor.transpose(b1ps, b1s, identity[:n1, :n1])
        b1t = const.tile([128, 2, JL, NHC], f32)
        nc.vector.tensor_copy(b1t.rearrange("p c j hc -> p (c j hc)"), b1ps)
        b2t = const.tile([128, 2, JL], f32)

        def bias2_stage():
            n2 = 2 * JL
            b2ps = ps2.tile([128, n2], f32, tag="ps2")
            nc.tensor.transpose(b2ps, b2s, identity[:n2, :n2])
            nc.vector.tensor_copy(b2t.rearrange("p c j -> p (c j)"), b2ps)

        def load_weights(j):
            w1t = w1p.tile([128, 2, H], f32, tag="w1t")  # [k, c, h]
            nc.scalar.dma_start(out=w1t, in_=w1[:, j].transpose([1, 0, 2]))
            w1n = w1np.tile([128, H], f32, tag="w1n")  # -w1[1,j]
            nc.vector.tensor_scalar_mul(w1n, w1t[:, 1], -1.0)
            w2t = w2p.tile([128, 2, NHC, K], f32, tag="w2t")  # [p, c, hc, k']
            for c in range(2):
                nc.scalar.dma_start(
                    out=w2t[:, c],
                    in_=w2[c, j].rearrange("(hc p) k -> p hc k", p=128),
                )
            # w2x[:,0,hc] = -w2[1];  w2x[:,1,hc] = w2[0]+w2[1]
            w2x = w2xp.tile([128, 2, NHC, K], f32, tag="w2x")
            nc.vector.tensor_scalar_mul(w2x[:, 0], w2t[:, 1], -1.0)
            nc.vector.tensor_add(w2x[:, 1], w2t[:, 0], w2t[:, 1])
            return w1t, w1n, w2t, w2x

        for j in range(JL):
            w1t, w1n, w2t, w2x = load_weights(j)
            xtr = xtp.tile([128, ROWS], f32, tag="xtr")
            nc.sync.dma_start(out=xtr, in_=xr[j])
            xti = xtp.tile([128, ROWS], f32, tag="xti")
            nc.sync.dma_start(out=xti, in_=xi[j])

            # --- layer 1 (w1 stationary; output transposed [h_chunk, rows]) ---
            o1r = o1p.tile([128, NHC, ROWS], f32, tag="o1r")
            o1i = o1p.tile([128, NHC, ROWS], f32, tag="o1i")
            for hc in range(NHC):
                hs = slice(hc * 128, (hc + 1) * 128)
                p1r = ps1.tile([128, ROWS], f32, tag="ps1")
                p1i = ps1.tile([128, ROWS], f32, tag="ps1")
                # w1[0] loaded once for both rhs streams
                nc.tensor.matmul(p1r, w1t[:, 0, hs], xtr, start=True, stop=False)
                nc.tensor.matmul(p1i, w1t[:, 0, hs], xti, start=True, stop=False)
                nc.tensor.matmul(p1r, w1n[:, hs], xti, start=False, stop=True)
                nc.tensor.matmul(p1i, w1t[:, 1, hs], xtr, start=False, stop=True)
                nc.scalar.activation(
                    o1r[:, hc], p1r, GELU, bias=b1t[:, 0, j, hc : hc + 1]
                )
                nc.scalar.activation(
                    o1i[:, hc], p1i, GELU, bias=b1t[:, 1, j, hc : hc + 1]
                )

            if j == 0:
                bias2_stage()

            # --- layer 2 (w2 stationary; output transposed [k', rows]) ---
            p2r = ps2.tile([128, ROWS], f32, tag="ps2")
            p2i = ps2.tile([128, ROWS], f32, tag="ps2")
            for hc in range(NHC):
                last = hc == NHC - 1
                nc.tensor.matmul(
                    p2r, w2t[:, 0, hc], o1r[:, hc], start=(hc == 0), stop=False
                )
                nc.tensor.matmul(
                    p2r, w2x[:, 0, hc], o1i[:, hc], start=False, stop=last
                )
                nc.tensor.matmul(
                    p2i, w2x[:, 1, hc], o1i[:, hc], start=(hc == 0), stop=last
                )

            # --- bias + drain + store (transposed; host fixes layout) ---
            otr = outp.tile([128, ROWS], f32, tag="ot")
            nc.vector.tensor_scalar_add(otr, p2r, b2t[:, 0, j : j + 1])
            nc.sync.dma_start(out=out[j, 0], in_=otr)
            oti = outp.tile([128, ROWS], f32, tag="ot")
            nc.vector.tensor_scalar_add(oti, p2i, b2t[:, 1, j : j + 1])
            nc.sync.dma_start(out=out[j, 1], in_=oti)

    if not nc.is_finalized():
        nc.finalize()
    return nc


def _shard_inputs(x_real, x_imag, w1, b1, w2, b2):
    in_maps = []
    for jg in range(NJG):
        for rg in range(NRG):
            js = slice(jg * JL, (jg + 1) * JL)
            bs = slice(rg * BL, (rg + 1) * BL)
            # [BL, I, JL, K] -> [JL, K, BL*I]: kernel wants x pre-transposed
            xr_s = np.ascontiguousarray(
                x_real[bs, :, js, :].transpose(2, 3, 0, 1).reshape(JL, K, ROWS)
            )
            xi_s = np.ascontiguousarray(
                x_imag[bs, :, js, :].transpose(2, 3, 0, 1).reshape(JL, K, ROWS)
            )
            in_maps.append(
                {
                    "xr": xr_s,
                    "xi": xi_s,
                    "w1": np.ascontiguousarray(w1[:, js]),
                    "b1": np.ascontiguousarray(b1[:, js]),
                    "w2": np.ascontiguousarray(w2[:, js]),
                    "b2": np.ascontiguousarray(b2[:, js]),
                }
            )
    return in_maps


def _gather(results):
    out = np.empty((B, I, J, K), np.complex64)
    idx = 0
    for jg in range(NJG):
        for rg in range(NRG):
            js = slice(jg * JL, (jg + 1) * JL)
            bs = slice(rg * BL, (rg + 1) * BL)
            o = np.asarray(results[idx]["out"], dtype=np.float32)  # [13,2,128,512]
            oc = (o[:, 0] + 1j * o[:, 1]).astype(np.complex64)  # [13,128,512]
            # [j, k, rows] -> [rows, j, k] -> [BL, I, JL, K]
            out[bs, :, js, :] = oc.transpose(2, 0, 1).reshape(BL, I, JL, K)
            idx += 1
    return out


def run(trace=False, **inputs):
    from concourse.bass_utils import run_bass_kernel_spmd

    if "nc" not in _cache:
        _cache["nc"] = _build_nc()
    in_maps = _shard_inputs(
        np.asarray(inputs["x_real"], np.float32),
        np.asarray(inputs["x_imag"], np.float32),
        np.asarray(inputs["w1"], np.float32),
        np.asarray(inputs["b1"], np.float32),
        np.asarray(inputs["w2"], np.float32),
        np.asarray(inputs["b2"], np.float32),
    )
    res = run_bass_kernel_spmd(_cache["nc"], in_maps, list(range(8)), trace=trace)
    return _gather(res.results), res


def kernel(**inputs):
    out, _ = run(trace=False, **inputs)
    return out


# revision 29
# speedup vs baseline: 1.5706x; 1.1494x over previous
"""Trainium2 Bass kernel for nn_MlpMixer_18966575579742.

Complex-valued per-frequency (j) MLP:
  o1r = gelu(xr@w1[0] - xi@w1[1] + b1[0]);  o1i = gelu(xi@w1[0] + xr@w1[1] + b1[1])
  o2r = o1r@w2[0] - o1i@w2[1] + b2[0];      o2i = o1i@w2[0] + o1i@w2[1] + b2[1]
  (note: o2i intentionally uses o1i with BOTH w2[0] and w2[1], as in the source)

Sharding over 8 cores: 2 j-halves (13 each) x 4 batch-quarters (B=32 -> 512 rows).
Per-core dataflow (all fp32; fp32 matmul = 2 HW passes at ~1.2 GHz):
  - PE-transpose x row-chunks into xT [k=128, rows=512] (SBUF via DVE copy)
  - L1 (w1 stationary, xT moving, N=512): o1T chunks [h_chunk=128, rows] in PSUM
  - exact-erf GELU + per-partition b1 bias on ScalarE (partitions = h)
  - L2 (w2 stationary, o1T moving, N=512): o2T [k'=128, rows] PSUM, accumulated
    via w2[0], -w2[1] (real) and w2[0]+w2[1] (imag)
  - DVE drains PSUM with fused per-partition b2 bias (partitions = k')
  - output stays transposed [j, c, k', rows]; host does the final
    transpose + complex interleave (cheap numpy ops on gathered results)
"""

import sys

if "/opt/trn_rl_repo" not in sys.path:
    sys.path.insert(0, "/opt/trn_rl_repo")

import numpy as np

B, I, J, K, F = 128, 16, 26, 128, 4
H = K * F  # 512
NJG = 2  # j groups
NRG = 4  # row (batch) groups
JL = J // NJG  # 13 j per core
BL = B // NRG  # 32 batches per core
ROWS = BL * I  # 512 rows per core
NHC = H // 128  # 4 h-chunks
NRC = ROWS // 128  # 4 row-chunks

_cache = {}


def _build_nc():
    from contextlib import ExitStack

    import concourse.bass as bass
    import concourse.mybir as mybir
    import concourse.tile as tile
    from concourse import bacc
    from concourse.masks import make_identity

    f32 = mybir.dt.float32
    nc = bacc.Bacc(None)

    # x arrives pre-transposed from the host: [j, k, rows]
    xr = nc.declare_dram_parameter("xr", [JL, K, ROWS], f32, isOutput=False)
    xi = nc.declare_dram_parameter("xi", [JL, K, ROWS], f32, isOutput=False)
    w1 = nc.declare_dram_parameter("w1", [2, JL, K, H], f32, isOutput=False)
    b1 = nc.declare_dram_parameter("b1", [2, JL, H], f32, isOutput=False)
    w2 = nc.declare_dram_parameter("w2", [2, JL, H, K], f32, isOutput=False)
    b2 = nc.declare_dram_parameter("b2", [2, JL, K], f32, isOutput=False)
    # transposed output: [j, c, k', rows]; host fixes layout
    out = nc.declare_dram_parameter("out", [JL, 2, K, ROWS], f32, isOutput=True)

    GELU = mybir.ActivationFunctionType.Gelu

    with tile.TileContext(nc) as tc, ExitStack() as ctx:
        const = ctx.enter_context(tc.tile_pool(name="const", bufs=1))
        w1p = ctx.enter_context(tc.tile_pool(name="w1p", bufs=3))
        w1np = ctx.enter_context(tc.tile_pool(name="w1np", bufs=2))
        w2p = ctx.enter_context(tc.tile_pool(name="w2p", bufs=3))
        w2xp = ctx.enter_context(tc.tile_pool(name="w2xp", bufs=2))
        xtp = ctx.enter_context(tc.tile_pool(name="xtp", bufs=3))
        o1p = ctx.enter_context(tc.tile_pool(name="o1p", bufs=2))
        cmb = ctx.enter_context(tc.tile_pool(name="cmb", bufs=2))
        outp = ctx.enter_context(tc.tile_pool(name="outp", bufs=4))
        ps1 = ctx.enter_context(tc.tile_pool(name="ps1", bufs=6, space="PSUM"))
        ps2 = ctx.enter_context(tc.tile_pool(name="ps2", bufs=2, space="PSUM"))

        identity = const.tile([128, 128], f32)
        make_identity(nc, identity)

        # biases: clean row-major staging DMA, then PE-transpose on chip.
        # b1s[(c j hc), p] rows are contiguous 512B; b1t[p, c, j, hc]
        b1s = const.tile([2 * JL * NHC, 128], f32)
        nc.gpsimd.dma_start(
            out=b1s, in_=b1.rearrange("c j (hc p) -> (c j hc) p", p=128)
        )
        b2s = const.tile([2 * JL, K], f32)
        nc.gpsimd.dma_start(out=b2s, in_=b2.rearrange("c j k -> (c j) k"))
        n1 = 2 * JL * NHC
        b1ps = ps2.tile([128, n1], f32, tag="ps2")
        nc.tensor.transpose(b1ps, b1s, identity[:n1, :n1])
        b1t = const.tile([128, 2, JL, NHC], f32)
        nc.vector.tensor_copy(b1t.rearrange("p c j hc -> p (c j hc)"), b1ps)
        b2t = const.tile([128, 2, JL], f32)

        def bias2_stage():
            n2 = 2 * JL
            b2ps = ps2.tile([128, n2], f32, tag="ps2")
            nc.tensor.transpose(b2ps, b2s, identity[:n2, :n2])
            nc.vector.tensor_copy(b2t.rearrange("p c j -> p (c j)"), b2ps)

        def load_weights(j):
            w1t = w1p.tile([128, 2, H], f32, tag="w1t")  # [k, c, h]
            nc.scalar.dma_start(out=w1t, in_=w1[:, j].transpose([1, 0, 2]))
            # Gauss 3-mult complex product weights:
            # w1g[:,0] = w1[1]-w1[0];  w1g[:,1] = w1[0]+w1[1]
            w1g = w1np.tile([128, 2, H], f32, tag="w1n")
            nc.vector.tensor_sub(w1g[:, 0], w1t[:, 1], w1t[:, 0])
            nc.vector.tensor_add(w1g[:, 1], w1t[:, 0], w1t[:, 1])
            w2t = w2p.tile([128, 2, NHC, K], f32, tag="w2t")  # [p, c, hc, k']
            for c in range(2):
                nc.scalar.dma_start(
                    out=w2t[:, c],
                    in_=w2[c, j].rearrange("(hc p) k -> p hc k", p=128),
                )
            # w2x[:,0,hc] = -w2[1];  w2x[:,1,hc] = w2[0]+w2[1]
            w2x = w2xp.tile([128, 2, NHC, K], f32, tag="w2x")
            nc.vector.tensor_scalar_mul(w2x[:, 0], w2t[:, 1], -1.0)
            nc.vector.tensor_add(w2x[:, 1], w2t[:, 0], w2t[:, 1])
            return w1t, w1g, w2t, w2x

        for j in range(JL):
            w1t, w1g, w2t, w2x = load_weights(j)
            xtr = xtp.tile([128, ROWS], f32, tag="xtr")
            nc.sync.dma_start(out=xtr, in_=xr[j])
            xti = xtp.tile([128, ROWS], f32, tag="xti")
            nc.sync.dma_start(out=xti, in_=xi[j])
            xsum = xtp.tile([128, ROWS], f32, tag="xsum")
            nc.vector.tensor_add(xsum, xtr, xti)

            # --- layer 1 via Gauss: t1=(xr+xi)@w1[0], t2=xr@(w1[1]-w1[0]),
            # t3=xi@(w1[0]+w1[1]);  o1r=gelu(t1-t3+b1r), o1i=gelu(t1+t2+b1i)
            o1r = o1p.tile([128, NHC, ROWS], f32, tag="o1r")
            o1i = o1p.tile([128, NHC, ROWS], f32, tag="o1i")
            for hc in range(NHC):
                hs = slice(hc * 128, (hc + 1) * 128)
                t1 = ps1.tile([128, ROWS], f32, tag="ps1")
                t2 = ps1.tile([128, ROWS], f32, tag="ps1")
                t3 = ps1.tile([128, ROWS], f32, tag="ps1")
                nc.tensor.matmul(t1, w1t[:, 0, hs], xsum, start=True, stop=True)
                nc.tensor.matmul(t2, w1g[:, 0, hs], xtr, start=True, stop=True)
                nc.tensor.matmul(t3, w1g[:, 1, hs], xti, start=True, stop=True)
                s1 = cmb.tile([128, ROWS], f32, tag="s1")
                nc.vector.tensor_copy(s1, t1)
                rp = cmb.tile([128, ROWS], f32, tag="rp")
                nc.vector.tensor_sub(rp, s1, t3)
                ip = cmb.tile([128, ROWS], f32, tag="ip")
                nc.vector.tensor_add(ip, s1, t2)
                nc.scalar.activation(
                    o1r[:, hc], rp, GELU, bias=b1t[:, 0, j, hc : hc + 1]
                )
                nc.scalar.activation(
                    o1i[:, hc], ip, GELU, bias=b1t[:, 1, j, hc : hc + 1]
                )

            if j == 0:
                bias2_stage()

            # --- layer 2 (w2 stationary; output transposed [k', rows]) ---
            p2r = ps2.tile([128, ROWS], f32, tag="ps2")
            p2i = ps2.tile([128, ROWS], f32, tag="ps2")
            for hc in range(NHC):
                last = hc == NHC - 1
                nc.tensor.matmul(
                    p2r, w2t[:, 0, hc], o1r[:, hc], start=(hc == 0), stop=False
                )
                nc.tensor.matmul(
                    p2r, w2x[:, 0, hc], o1i[:, hc], start=False, stop=last
                )
                nc.tensor.matmul(
                    p2i, w2x[:, 1, hc], o1i[:, hc], start=(hc == 0), stop=last
                )

            # --- bias + drain + store (transposed; host fixes layout) ---
            otr = outp.tile([128, ROWS], f32, tag="ot")
            nc.vector.tensor_scalar_add(otr, p2r, b2t[:, 0, j : j + 1])
            nc.sync.dma_start(out=out[j, 0], in_=otr)
            oti = outp.tile([128, ROWS], f32, tag="ot")
            nc.vector.tensor_scalar_add(oti, p2i, b2t[:, 1, j : j + 1])
            nc.sync.dma_start(out=out[j, 1], in_=oti)

    if not nc.is_finalized():
        nc.finalize()
    return nc


def _shard_inputs(x_real, x_imag, w1, b1, w2, b2):
    in_maps = []
    for jg in range(NJG):
        for rg in range(NRG):
            js = slice(jg * JL, (jg + 1) * JL)
            bs = slice(rg * BL, (rg + 1) * BL)
            # [BL, I, JL, K] -> [JL, K, BL*I]: kernel wants x pre-transposed
            xr_s = np.ascontiguousarray(
                x_real[bs, :, js, :].transpose(2, 3, 0, 1).reshape(JL, K, ROWS)
            )
            xi_s = np.ascontiguousarray(
                x_imag[bs, :, js, :].transpose(2, 3, 0, 1).reshape(JL, K, ROWS)
            )
            in_maps.append(
                {
                    "xr": xr_s,
                    "xi": xi_s,
                    "w1": np.ascontiguousarray(w1[:, js]),
                    "b1": np.ascontiguousarray(b1[:, js]),
                    "w2": np.ascontiguousarray(w2[:, js]),
                    "b2": np.ascontiguousarray(b2[:, js]),
                }
            )
    return in_maps


def _gather(results):
    out = np.empty((B, I, J, K), np.complex64)
    idx = 0
    for jg in range(NJG):
        for rg in range(NRG):
            js = slice(jg * JL, (jg + 1) * JL)
            bs = slice(rg * BL, (rg + 1) * BL)
            o = np.asarray(results[idx]["out"], dtype=np.float32)  # [13,2,128,512]
            oc = (o[:, 0] + 1j * o[:, 1]).astype(np.complex64)  # [13,128,512]
            # [j, k, rows] -> [rows, j, k] -> [BL, I, JL, K]
            out[bs, :, js, :] = oc.transpose(2, 0, 1).reshape(BL, I, JL, K)
            idx += 1
    return out


def run(trace=False, **inputs):
    from concourse.bass_utils import run_bass_kernel_spmd

    if "nc" not in _cache:
        _cache["nc"] = _build_nc()
    in_maps = _shard_inputs(
        np.asarray(inputs["x_real"], np.float32),
        np.asarray(inputs["x_imag"], np.float32),
        np.asarray(inputs["w1"], np.float32),
        np.asarray(inputs["b1"], np.float32),
        np.asarray(inputs["w2"], np.float32),
        np.asarray(inputs["b2"], np.float32),
    )
    res = run_bass_kernel_spmd(_cache["nc"], in_maps, list(range(8)), trace=trace)
    return _gather(res.results), res


def kernel(**inputs):
    out, _ = run(trace=False, **inputs)
    return out


# revision 32
# speedup vs baseline: 1.5764x; 1.0037x over previous
"""Trainium2 Bass kernel for nn_MlpMixer_18966575579742.

Complex-valued per-frequency (j) MLP:
  o1r = gelu(xr@w1[0] - xi@w1[1] + b1[0]);  o1i = gelu(xi@w1[0] + xr@w1[1] + b1[1])
  o2r = o1r@w2[0] - o1i@w2[1] + b2[0];      o2i = o1i@w2[0] + o1i@w2[1] + b2[1]
  (note: o2i intentionally uses o1i with BOTH w2[0] and w2[1], as in the source)

Sharding over 8 cores: 2 j-halves (13 each) x 4 batch-quarters (B=32 -> 512 rows).
Per-core dataflow (all fp32; fp32 matmul = 2 HW passes at ~1.2 GHz):
  - PE-transpose x row-chunks into xT [k=128, rows=512] (SBUF via DVE copy)
  - L1 (w1 stationary, xT moving, N=512): o1T chunks [h_chunk=128, rows] in PSUM
  - exact-erf GELU + per-partition b1 bias on ScalarE (partitions = h)
  - L2 (w2 stationary, o1T moving, N=512): o2T [k'=128, rows] PSUM, accumulated
    via w2[0], -w2[1] (real) and w2[0]+w2[1] (imag)
  - DVE drains PSUM with fused per-partition b2 bias (partitions = k')
  - output stays transposed [j, c, k', rows]; host does the final
    transpose + complex interleave (cheap numpy ops on gathered results)
"""

import sys

if "/opt/trn_rl_repo" not in sys.path:
    sys.path.insert(0, "/opt/trn_rl_repo")

import numpy as np

B, I, J, K, F = 128, 16, 26, 128, 4
H = K * F  # 512
NJG = 2  # j groups
NRG = 4  # row (batch) groups
JL = J // NJG  # 13 j per core
BL = B // NRG  # 32 batches per core
ROWS = BL * I  # 512 rows per core
NHC = H // 128  # 4 h-chunks
NRC = ROWS // 128  # 4 row-chunks

_cache = {}


def _build_nc():
    from contextlib import ExitStack

    import concourse.bass as bass
    import concourse.mybir as mybir
    import concourse.tile as tile
    from concourse import bacc
    from concourse.masks import make_identity

    f32 = mybir.dt.float32
    nc = bacc.Bacc(None)

    # x arrives pre-transposed from the host: [j, k, rows]; xs = xr + xi
    xr = nc.declare_dram_parameter("xr", [JL, K, ROWS], f32, isOutput=False)
    xi = nc.declare_dram_parameter("xi", [JL, K, ROWS], f32, isOutput=False)
    xs = nc.declare_dram_parameter("xs", [JL, K, ROWS], f32, isOutput=False)
    w1 = nc.declare_dram_parameter("w1", [2, JL, K, H], f32, isOutput=False)
    b1 = nc.declare_dram_parameter("b1", [2, JL, H], f32, isOutput=False)
    w2 = nc.declare_dram_parameter("w2", [2, JL, H, K], f32, isOutput=False)
    b2 = nc.declare_dram_parameter("b2", [2, JL, K], f32, isOutput=False)
    # transposed output: [j, c, k', rows]; host fixes layout
    out = nc.declare_dram_parameter("out", [JL, 2, K, ROWS], f32, isOutput=True)

    GELU = mybir.ActivationFunctionType.Gelu

    with tile.TileContext(nc) as tc, ExitStack() as ctx:
        const = ctx.enter_context(tc.tile_pool(name="const", bufs=1))
        w1p = ctx.enter_context(tc.tile_pool(name="w1p", bufs=3))
        w1np = ctx.enter_context(tc.tile_pool(name="w1np", bufs=2))
        w2p = ctx.enter_context(tc.tile_pool(name="w2p", bufs=3))
        w2xp = ctx.enter_context(tc.tile_pool(name="w2xp", bufs=2))
        xtp = ctx.enter_context(tc.tile_pool(name="xtp", bufs=3))
        o1p = ctx.enter_context(tc.tile_pool(name="o1p", bufs=2))
        cmb = ctx.enter_context(tc.tile_pool(name="cmb", bufs=2))
        outp = ctx.enter_context(tc.tile_pool(name="outp", bufs=4))
        ps1 = ctx.enter_context(tc.tile_pool(name="ps1", bufs=6, space="PSUM"))
        ps2 = ctx.enter_context(tc.tile_pool(name="ps2", bufs=2, space="PSUM"))

        identity = const.tile([128, 128], f32)
        make_identity(nc, identity)

        # biases: clean row-major staging DMA, then PE-transpose on chip.
        # b1s[(c j hc), p] rows are contiguous 512B; b1t[p, c, j, hc]
        b1s = const.tile([2 * JL * NHC, 128], f32)
        nc.gpsimd.dma_start(
            out=b1s, in_=b1.rearrange("c j (hc p) -> (c j hc) p", p=128)
        )
        b2s = const.tile([2 * JL, K], f32)
        nc.gpsimd.dma_start(out=b2s, in_=b2.rearrange("c j k -> (c j) k"))
        n1 = 2 * JL * NHC
        b1ps = ps2.tile([128, n1], f32, tag="ps2")
        nc.tensor.transpose(b1ps, b1s, identity[:n1, :n1])
        b1t = const.tile([128, 2, JL, NHC], f32)
        nc.vector.tensor_copy(b1t.rearrange("p c j hc -> p (c j hc)"), b1ps)
        b2t = const.tile([128, 2, JL], f32)

        def bias2_stage():
            n2 = 2 * JL
            b2ps = ps2.tile([128, n2], f32, tag="ps2")
            nc.tensor.transpose(b2ps, b2s, identity[:n2, :n2])
            nc.vector.tensor_copy(b2t.rearrange("p c j -> p (c j)"), b2ps)

        def load_weights(j):
            w1t = w1p.tile([128, 2, H], f32, tag="w1t")  # [k, c, h]
            nc.scalar.dma_start(out=w1t, in_=w1[:, j].transpose([1, 0, 2]))
            # Gauss 3-mult complex product weights:
            # w1g[:,0] = w1[1]-w1[0];  w1g[:,1] = w1[0]+w1[1]
            w1g = w1np.tile([128, 2, H], f32, tag="w1n")
            nc.vector.tensor_sub(w1g[:, 0], w1t[:, 1], w1t[:, 0])
            nc.vector.tensor_add(w1g[:, 1], w1t[:, 0], w1t[:, 1])
            w2t = w2p.tile([128, 2, NHC, K], f32, tag="w2t")  # [p, c, hc, k']
            for c in range(2):
                nc.scalar.dma_start(
                    out=w2t[:, c],
                    in_=w2[c, j].rearrange("(hc p) k -> p hc k", p=128),
                )
            # w2x[:,0,hc] = -w2[1];  w2x[:,1,hc] = w2[0]+w2[1]
            w2x = w2xp.tile([128, 2, NHC, K], f32, tag="w2x")
            nc.vector.tensor_scalar_mul(w2x[:, 0], w2t[:, 1], -1.0)
            nc.vector.tensor_add(w2x[:, 1], w2t[:, 0], w2t[:, 1])
            return w1t, w1g, w2t, w2x

        for j in range(JL):
            w1t, w1g, w2t, w2x = load_weights(j)
            xtr = xtp.tile([128, ROWS], f32, tag="xtr")
            nc.sync.dma_start(out=xtr, in_=xr[j])
            xti = xtp.tile([128, ROWS], f32, tag="xti")
            nc.sync.dma_start(out=xti, in_=xi[j])
            xsum = xtp.tile([128, ROWS], f32, tag="xsum")
            nc.sync.dma_start(out=xsum, in_=xs[j])

            # --- layer 1 via Gauss: t1=(xr+xi)@w1[0], t2=xr@(w1[1]-w1[0]),
            # t3=xi@(w1[0]+w1[1]);  o1r=gelu(t1-t3+b1r), o1i=gelu(t1+t2+b1i)
            o1r = o1p.tile([128, NHC, ROWS], f32, tag="o1r")
            o1i = o1p.tile([128, NHC, ROWS], f32, tag="o1i")
            for hc in range(NHC):
                hs = slice(hc * 128, (hc + 1) * 128)
                t1 = ps1.tile([128, ROWS], f32, tag="ps1")
                t2 = ps1.tile([128, ROWS], f32, tag="ps1")
                t3 = ps1.tile([128, ROWS], f32, tag="ps1")
                nc.tensor.matmul(t1, w1t[:, 0, hs], xsum, start=True, stop=True)
                nc.tensor.matmul(t2, w1g[:, 0, hs], xtr, start=True, stop=True)
                nc.tensor.matmul(t3, w1g[:, 1, hs], xti, start=True, stop=True)
                s1 = cmb.tile([128, ROWS], f32, tag="s1")
                nc.vector.tensor_copy(s1, t1)
                rp = cmb.tile([128, ROWS], f32, tag="rp")
                nc.vector.tensor_sub(rp, s1, t3)
                ip = cmb.tile([128, ROWS], f32, tag="ip")
                nc.vector.tensor_add(ip, s1, t2)
                nc.scalar.activation(
                    o1r[:, hc], rp, GELU, bias=b1t[:, 0, j, hc : hc + 1]
                )
                nc.scalar.activation(
                    o1i[:, hc], ip, GELU, bias=b1t[:, 1, j, hc : hc + 1]
                )

            if j == 0:
                bias2_stage()

            # --- layer 2 (w2 stationary; output transposed [k', rows]) ---
            p2r = ps2.tile([128, ROWS], f32, tag="ps2")
            p2i = ps2.tile([128, ROWS], f32, tag="ps2")
            for hc in range(NHC):
                last = hc == NHC - 1
                nc.tensor.matmul(
                    p2r, w2t[:, 0, hc], o1r[:, hc], start=(hc == 0), stop=False
                )
                nc.tensor.matmul(
                    p2r, w2x[:, 0, hc], o1i[:, hc], start=False, stop=last
                )
                nc.tensor.matmul(
                    p2i, w2x[:, 1, hc], o1i[:, hc], start=(hc == 0), stop=last
                )

            # --- bias + drain + store (transposed; host fixes layout) ---
            otr = outp.tile([128, ROWS], f32, tag="ot")
            nc.vector.tensor_scalar_add(otr, p2r, b2t[:, 0, j : j + 1])
            nc.sync.dma_start(out=out[j, 0], in_=otr)
            oti = outp.tile([128, ROWS], f32, tag="ot")
            nc.vector.tensor_scalar_add(oti, p2i, b2t[:, 1, j : j + 1])
            nc.sync.dma_start(out=out[j, 1], in_=oti)

    if not nc.is_finalized():
        nc.finalize()
    return nc


def _shard_inputs(x_real, x_imag, w1, b1, w2, b2):
    in_maps = []
    for jg in range(NJG):
        for rg in range(NRG):
            js = slice(jg * JL, (jg + 1) * JL)
            bs = slice(rg * BL, (rg + 1) * BL)
            # [BL, I, JL, K] -> [JL, K, BL*I]: kernel wants x pre-transposed
            xr_s = np.ascontiguousarray(
                x_real[bs, :, js, :].transpose(2, 3, 0, 1).reshape(JL, K, ROWS)
            )
            xi_s = np.ascontiguousarray(
                x_imag[bs, :, js, :].transpose(2, 3, 0, 1).reshape(JL, K, ROWS)
            )
            in_maps.append(
                {
                    "xr": xr_s,
                    "xi": xi_s,
                    "xs": xr_s + xi_s,
                    "w1": np.ascontiguousarray(w1[:, js]),
                    "b1": np.ascontiguousarray(b1[:, js]),
                    "w2": np.ascontiguousarray(w2[:, js]),
                    "b2": np.ascontiguousarray(b2[:, js]),
                }
            )
    return in_maps


def _gather(results):
    out = np.empty((B, I, J, K), np.complex64)
    idx = 0
    for jg in range(NJG):
        for rg in range(NRG):
            js = slice(jg * JL, (jg + 1) * JL)
            bs = slice(rg * BL, (rg + 1) * BL)
            o = np.asarray(results[idx]["out"], dtype=np.float32)  # [13,2,128,512]
            oc = (o[:, 0] + 1j * o[:, 1]).astype(np.complex64)  # [13,128,512]
            # [j, k, rows] -> [rows, j, k] -> [BL, I, JL, K]
            out[bs, :, js, :] = oc.transpose(2, 0, 1).reshape(BL, I, JL, K)
            idx += 1
    return out


def run(trace=False, **inputs):
    from concourse.bass_utils import run_bass_kernel_spmd

    if "nc" not in _cache:
        _cache["nc"] = _build_nc()
    in_maps = _shard_inputs(
        np.asarray(inputs["x_real"], np.float32),
        np.asarray(inputs["x_imag"], np.float32),
        np.asarray(inputs["w1"], np.float32),
        np.asarray(inputs["b1"], np.float32),
        np.asarray(inputs["w2"], np.float32),
        np.asarray(inputs["b2"], np.float32),
    )
    res = run_bass_kernel_spmd(_cache["nc"], in_maps, list(range(8)), trace=trace)
    return _gather(res.results), res


def kernel(**inputs):
    out, _ = run(trace=False, **inputs)
    return out


# revision 34
# speedup vs baseline: 1.5818x; 1.0034x over previous
"""Trainium2 Bass kernel for nn_MlpMixer_18966575579742.

Complex-valued per-frequency (j) MLP:
  o1r = gelu(xr@w1[0] - xi@w1[1] + b1[0]);  o1i = gelu(xi@w1[0] + xr@w1[1] + b1[1])
  o2r = o1r@w2[0] - o1i@w2[1] + b2[0];      o2i = o1i@w2[0] + o1i@w2[1] + b2[1]
  (note: o2i intentionally uses o1i with BOTH w2[0] and w2[1], as in the source)

Sharding over 8 cores: 2 j-halves (13 each) x 4 batch-quarters (B=32 -> 512 rows).
Per-core dataflow (all fp32; fp32 matmul = 2 HW passes at ~1.2 GHz):
  - PE-transpose x row-chunks into xT [k=128, rows=512] (SBUF via DVE copy)
  - L1 (w1 stationary, xT moving, N=512): o1T chunks [h_chunk=128, rows] in PSUM
  - exact-erf GELU + per-partition b1 bias on ScalarE (partitions = h)
  - L2 (w2 stationary, o1T moving, N=512): o2T [k'=128, rows] PSUM, accumulated
    via w2[0], -w2[1] (real) and w2[0]+w2[1] (imag)
  - DVE drains PSUM with fused per-partition b2 bias (partitions = k')
  - output stays transposed [j, c, k', rows]; host does the final
    transpose + complex interleave (cheap numpy ops on gathered results)
"""

import sys

if "/opt/trn_rl_repo" not in sys.path:
    sys.path.insert(0, "/opt/trn_rl_repo")

import numpy as np

B, I, J, K, F = 128, 16, 26, 128, 4
H = K * F  # 512
NJG = 2  # j groups
NRG = 4  # row (batch) groups
JL = J // NJG  # 13 j per core
BL = B // NRG  # 32 batches per core
ROWS = BL * I  # 512 rows per core
NHC = H // 128  # 4 h-chunks
NRC = ROWS // 128  # 4 row-chunks

_cache = {}


def _build_nc():
    from contextlib import ExitStack

    import concourse.bass as bass
    import concourse.mybir as mybir
    import concourse.tile as tile
    from concourse import bacc
    from concourse.masks import make_identity

    f32 = mybir.dt.float32
    nc = bacc.Bacc(None)

    # x arrives pre-transposed from the host: [j, k, rows]; xs = xr + xi
    xr = nc.declare_dram_parameter("xr", [JL, K, ROWS], f32, isOutput=False)
    xi = nc.declare_dram_parameter("xi", [JL, K, ROWS], f32, isOutput=False)
    xs = nc.declare_dram_parameter("xs", [JL, K, ROWS], f32, isOutput=False)
    w1 = nc.declare_dram_parameter("w1", [2, JL, K, H], f32, isOutput=False)
    b1 = nc.declare_dram_parameter("b1", [2, JL, H], f32, isOutput=False)
    w2 = nc.declare_dram_parameter("w2", [2, JL, H, K], f32, isOutput=False)
    b2 = nc.declare_dram_parameter("b2", [2, JL, K], f32, isOutput=False)
    # transposed output: [j, c, k', rows]; host fixes layout
    out = nc.declare_dram_parameter("out", [JL, 2, K, ROWS], f32, isOutput=True)

    GELU = mybir.ActivationFunctionType.Gelu

    with tile.TileContext(nc) as tc, ExitStack() as ctx:
        const = ctx.enter_context(tc.tile_pool(name="const", bufs=1))
        w1p = ctx.enter_context(tc.tile_pool(name="w1p", bufs=3))
        w1np = ctx.enter_context(tc.tile_pool(name="w1np", bufs=2))
        w2p = ctx.enter_context(tc.tile_pool(name="w2p", bufs=3))
        w2xp = ctx.enter_context(tc.tile_pool(name="w2xp", bufs=2))
        xtp = ctx.enter_context(tc.tile_pool(name="xtp", bufs=3))
        o1p = ctx.enter_context(tc.tile_pool(name="o1p", bufs=2))
        cmb = ctx.enter_context(tc.tile_pool(name="cmb", bufs=2))
        outp = ctx.enter_context(tc.tile_pool(name="outp", bufs=4))
        ps1 = ctx.enter_context(tc.tile_pool(name="ps1", bufs=6, space="PSUM"))
        ps2 = ctx.enter_context(tc.tile_pool(name="ps2", bufs=2, space="PSUM"))

        identity = const.tile([128, 128], f32)
        make_identity(nc, identity)

        # biases: clean row-major staging DMA, then PE-transpose on chip.
        # b1s[(c j hc), p] rows are contiguous 512B; b1t[p, c, j, hc]
        b1s = const.tile([2 * JL * NHC, 128], f32)
        nc.gpsimd.dma_start(
            out=b1s, in_=b1.rearrange("c j (hc p) -> (c j hc) p", p=128)
        )
        b2s = const.tile([2 * JL, K], f32)
        nc.gpsimd.dma_start(out=b2s, in_=b2.rearrange("c j k -> (c j) k"))
        b1t = const.tile([128, 2, JL, NHC], f32)
        b2t = const.tile([128, 2, JL], f32)

        def bias1_stage():
            n1 = 2 * JL * NHC
            b1ps = ps2.tile([128, n1], f32, tag="ps2")
            nc.tensor.transpose(b1ps, b1s, identity[:n1, :n1])
            nc.vector.tensor_copy(b1t.rearrange("p c j hc -> p (c j hc)"), b1ps)

        def bias2_stage():
            n2 = 2 * JL
            b2ps = ps2.tile([128, n2], f32, tag="ps2")
            nc.tensor.transpose(b2ps, b2s, identity[:n2, :n2])
            nc.vector.tensor_copy(b2t.rearrange("p c j -> p (c j)"), b2ps)

        def load_weights(j):
            w1t = w1p.tile([128, 2, H], f32, tag="w1t")  # [k, c, h]
            nc.scalar.dma_start(out=w1t, in_=w1[:, j].transpose([1, 0, 2]))
            # Gauss 3-mult complex product weights:
            # w1g[:,0] = w1[1]-w1[0];  w1g[:,1] = w1[0]+w1[1]
            w1g = w1np.tile([128, 2, H], f32, tag="w1n")
            nc.vector.tensor_sub(w1g[:, 0], w1t[:, 1], w1t[:, 0])
            nc.vector.tensor_add(w1g[:, 1], w1t[:, 0], w1t[:, 1])
            w2t = w2p.tile([128, 2, NHC, K], f32, tag="w2t")  # [p, c, hc, k']
            for c in range(2):
                nc.scalar.dma_start(
                    out=w2t[:, c],
                    in_=w2[c, j].rearrange("(hc p) k -> p hc k", p=128),
                )
            # w2x[:,0,hc] = -w2[1];  w2x[:,1,hc] = w2[0]+w2[1]
            w2x = w2xp.tile([128, 2, NHC, K], f32, tag="w2x")
            nc.vector.tensor_scalar_mul(w2x[:, 0], w2t[:, 1], -1.0)
            nc.vector.tensor_add(w2x[:, 1], w2t[:, 0], w2t[:, 1])
            return w1t, w1g, w2t, w2x

        for j in range(JL):
            w1t, w1g, w2t, w2x = load_weights(j)
            xtr = xtp.tile([128, ROWS], f32, tag="xtr")
            nc.sync.dma_start(out=xtr, in_=xr[j])
            xti = xtp.tile([128, ROWS], f32, tag="xti")
            nc.sync.dma_start(out=xti, in_=xi[j])
            xsum = xtp.tile([128, ROWS], f32, tag="xsum")
            nc.sync.dma_start(out=xsum, in_=xs[j])

            # --- layer 1 via Gauss: t1=(xr+xi)@w1[0], t2=xr@(w1[1]-w1[0]),
            # t3=xi@(w1[0]+w1[1]);  o1r=gelu(t1-t3+b1r), o1i=gelu(t1+t2+b1i)
            o1r = o1p.tile([128, NHC, ROWS], f32, tag="o1r")
            o1i = o1p.tile([128, NHC, ROWS], f32, tag="o1i")
            for hc in range(NHC):
                hs = slice(hc * 128, (hc + 1) * 128)
                t1 = ps1.tile([128, ROWS], f32, tag="ps1")
                t2 = ps1.tile([128, ROWS], f32, tag="ps1")
                t3 = ps1.tile([128, ROWS], f32, tag="ps1")
                nc.tensor.matmul(t1, w1t[:, 0, hs], xsum, start=True, stop=True)
                nc.tensor.matmul(t2, w1g[:, 0, hs], xtr, start=True, stop=True)
                nc.tensor.matmul(t3, w1g[:, 1, hs], xti, start=True, stop=True)
                if j == 0 and hc == 0:
                    # fills the PE pipe while the first GELU waits on b1t
                    bias1_stage()
                s1 = cmb.tile([128, ROWS], f32, tag="s1")
                nc.vector.tensor_copy(s1, t1)
                rp = cmb.tile([128, ROWS], f32, tag="rp")
                nc.vector.tensor_sub(rp, s1, t3)
                ip = cmb.tile([128, ROWS], f32, tag="ip")
                nc.vector.tensor_add(ip, s1, t2)
                nc.scalar.activation(
                    o1r[:, hc], rp, GELU, bias=b1t[:, 0, j, hc : hc + 1]
                )
                nc.scalar.activation(
                    o1i[:, hc], ip, GELU, bias=b1t[:, 1, j, hc : hc + 1]
                )

            if j == 0:
                bias2_stage()

            # --- layer 2 (w2 stationary; output transposed [k', rows]) ---
            p2r = ps2.tile([128, ROWS], f32, tag="ps2")
            p2i = ps2.tile([128, ROWS], f32, tag="ps2")
            for hc in range(NHC):
                last = hc == NHC - 1
                nc.tensor.matmul(
                    p2r, w2t[:, 0, hc], o1r[:, hc], start=(hc == 0), stop=False
                )
                nc.tensor.matmul(
                    p2r, w2x[:, 0, hc], o1i[:, hc], start=False, stop=last
                )
                nc.tensor.matmul(
                    p2i, w2x[:, 1, hc], o1i[:, hc], start=(hc == 0), stop=last
                )

            # --- bias + drain + store (transposed; host fixes layout) ---
            otr = outp.tile([128, ROWS], f32, tag="ot")
            nc.vector.tensor_scalar_add(otr, p2r, b2t[:, 0, j : j + 1])
            nc.sync.dma_start(out=out[j, 0], in_=otr)
            oti = outp.tile([128, ROWS], f32, tag="ot")
            nc.vector.tensor_scalar_add(oti, p2i, b2t[:, 1, j : j + 1])
            nc.sync.dma_start(out=out[j, 1], in_=oti)

    if not nc.is_finalized():
        nc.finalize()
    return nc


def _shard_inputs(x_real, x_imag, w1, b1, w2, b2):
    in_maps = []
    for jg in range(NJG):
        for rg in range(NRG):
            js = slice(jg * JL, (jg + 1) * JL)
            bs = slice(rg * BL, (rg + 1) * BL)
            # [BL, I, JL, K] -> [JL, K, BL*I]: kernel wants x pre-transposed
            xr_s = np.ascontiguousarray(
                x_real[bs, :, js, :].transpose(2, 3, 0, 1).reshape(JL, K, ROWS)
            )
            xi_s = np.ascontiguousarray(
                x_imag[bs, :, js, :].transpose(2, 3, 0, 1).reshape(JL, K, ROWS)
            )
            in_maps.append(
                {
                    "xr": xr_s,
                    "xi": xi_s,
                    "xs": xr_s + xi_s,
                    "w1": np.ascontiguousarray(w1[:, js]),
                    "b1": np.ascontiguousarray(b1[:, js]),
                    "w2": np.ascontiguousarray(w2[:, js]),
                    "b2": np.ascontiguousarray(b2[:, js]),
                }
            )
    return in_maps


def _gather(results):
    out = np.empty((B, I, J, K), np.complex64)
    idx = 0
    for jg in range(NJG):
        for rg in range(NRG):
            js = slice(jg * JL, (jg + 1) * JL)
            bs = slice(rg * BL, (rg + 1) * BL)
            o = np.asarray(results[idx]["out"], dtype=np.float32)  # [13,2,128,512]
            oc = (o[:, 0] + 1j * o[:, 1]).astype(np.complex64)  # [13,128,512]
            # [j, k, rows] -> [rows, j, k] -> [BL, I, JL, K]
            out[bs, :, js, :] = oc.transpose(2, 0, 1).reshape(BL, I, JL, K)
            idx += 1
    return out


def run(trace=False, **inputs):
    from concourse.bass_utils import run_bass_kernel_spmd

    if "nc" not in _cache:
        _cache["nc"] = _build_nc()
    in_maps = _shard_inputs(
        np.asarray(inputs["x_real"], np.float32),
        np.asarray(inputs["x_imag"], np.float32),
        np.asarray(inputs["w1"], np.float32),
        np.asarray(inputs["b1"], np.float32),
        np.asarray(inputs["w2"], np.float32),
        np.asarray(inputs["b2"], np.float32),
    )
    res = run_bass_kernel_spmd(_cache["nc"], in_maps, list(range(8)), trace=trace)
    return _gather(res.results), res


def kernel(**inputs):
    out, _ = run(trace=False, **inputs)
    return out
